# revision 1
# baseline (speedup 1.0000x reference)
"""GCN (CGCN) forward kernel for Trainium2, data-parallel over batch on 8 NeuronCores.

Per core (one batch sample):
  d      = adj.sum(-1) + 1 ;  dinv = d**-0.5
  mv     = relu(x @ Wm + bm)
  t1     = adj @ (dinv*mv) + (dinv*mv)          # A @ Dinv @ mv  with A = adj + I
  hid    = relu(dinv*(t1 @ W1) + b1)
  t2     = adj @ (dinv*hid) + (dinv*hid)
  out    = relu(dinv*(t2 @ W2) + b2)

adj is streamed from HBM exactly once; its transpose (needed because the PE
contracts over the partition axis) is built on-chip with identity matmuls and
kept resident in SBUF.
"""

import numpy as np

B, N, IN_DIM, HID_DIM, OUT_DIM = 8, 2048, 256, 256, 128
P = 128


def _build_module(n=N, din=IN_DIM, dhid=HID_DIM, dout=OUT_DIM, astage_bufs=3):
    from contextlib import ExitStack

    import concourse.mybir as mybir
    import concourse.tile as tile
    from concourse import bacc

    f32 = mybir.dt.float32
    AX = mybir.AxisListType
    OP = mybir.AluOpType
    AF = mybir.ActivationFunctionType

    T = n // P            # node tiles
    KD = din // P         # input-feature blocks
    KH = dhid // P        # hidden-feature blocks
    CH = min(512, n)      # A-product chunk width (one PSUM bank)
    NCH = n // CH
    DVE_COLS = n // 2     # row-sum split between VectorE and ScalarE

    nc = bacc.Bacc(None, target_bir_lowering=False, dynamic_dma_scratch_size=2048)

    adj_d = nc.declare_dram_parameter("adj", [n, n], f32, isOutput=False)
    x_d = nc.declare_dram_parameter("x", [n, din], f32, isOutput=False)
    wm_d = nc.declare_dram_parameter("wm", [din, din], f32, isOutput=False)
    w1_d = nc.declare_dram_parameter("w1", [din, dhid], f32, isOutput=False)
    w2_d = nc.declare_dram_parameter("w2", [dhid, dout], f32, isOutput=False)
    bm_d = nc.declare_dram_parameter("bm", [P, din], f32, isOutput=False)
    b1_d = nc.declare_dram_parameter("b1", [P, dhid], f32, isOutput=False)
    b2_d = nc.declare_dram_parameter("b2", [P, dout], f32, isOutput=False)
    eye_d = nc.declare_dram_parameter("eye", [P, P], f32, isOutput=False)
    mv_d = nc.declare_dram_parameter("mv", [n, din], f32, isOutput=True)
    hid_d = nc.declare_dram_parameter("hid", [n, dhid], f32, isOutput=True)
    out_d = nc.declare_dram_parameter("out", [n, dout], f32, isOutput=True)

    with tile.TileContext(nc) as tc:
        with ExitStack() as ctx:
            persist = ctx.enter_context(tc.tile_pool(name="persist", bufs=1))
            astage = ctx.enter_context(tc.tile_pool(name="astage", bufs=astage_bufs))
            small = ctx.enter_context(tc.tile_pool(name="small", bufs=3))
            feat = ctx.enter_context(tc.tile_pool(name="feat", bufs=2))
            ps = ctx.enter_context(tc.tile_pool(name="ps", bufs=2, space="PSUM"))

            # ---- constants -------------------------------------------------
            eye = persist.tile([P, P], f32, name="eye", tag="eye")
            nc.sync.dma_start(out=eye, in_=eye_d[:, :])
            wm_sb, w1_sb, w2_sb = [], [], []
            for k in range(KD):
                t = persist.tile([P, din], f32, name=f"wm{k}", tag=f"wm{k}")
                nc.sync.dma_start(out=t, in_=wm_d[k * P:(k + 1) * P, :])
                wm_sb.append(t)
            for k in range(KD):
                t = persist.tile([P, dhid], f32, name=f"w1_{k}", tag=f"w1_{k}")
                nc.sync.dma_start(out=t, in_=w1_d[k * P:(k + 1) * P, :])
                w1_sb.append(t)
            for k in range(KH):
                t = persist.tile([P, dout], f32, name=f"w2_{k}", tag=f"w2_{k}")
                nc.sync.dma_start(out=t, in_=w2_d[k * P:(k + 1) * P, :])
                w2_sb.append(t)
            bm_sb = persist.tile([P, din], f32, name="bm_sb", tag="bm_sb")
            nc.sync.dma_start(out=bm_sb, in_=bm_d[:, :])
            b1_sb = persist.tile([P, dhid], f32, name="b1_sb", tag="b1_sb")
            nc.sync.dma_start(out=b1_sb, in_=b1_d[:, :])
            b2_sb = persist.tile([P, dout], f32, name="b2_sb", tag="b2_sb")
            nc.sync.dma_start(out=b2_sb, in_=b2_d[:, :])

            d_sb = persist.tile([P, T], f32, name="d_sb", tag="d_sb")
            da_sb = persist.tile([P, T], f32, name="da_sb", tag="da_sb")
            d2_sb = persist.tile([P, T], f32, name="d2_sb", tag="d2_sb")
            srt_sb = persist.tile([P, T], f32, name="srt_sb", tag="srt_sb")
            dinv = persist.tile([P, T], f32, name="dinv", tag="dinv")

            # ---- x transpose (xT[k][:, i*P:(i+1)*P] = x[iP:(i+1)P, kP:(k+1)P].T)
            xT = [feat.tile([P, n], f32, name=f"xT{k}", tag="featbig") for k in range(KD)]
            for i in range(T):
                xt = small.tile([P, din], f32, name="xt", tag="xt")
                nc.sync.dma_start(out=xt, in_=x_d[i * P:(i + 1) * P, :])
                for k in range(KD):
                    ptx = ps.tile([P, P], f32, name="ptx", tag="ptrans", bufs=2)
                    nc.tensor.matmul(ptx, lhsT=xt[:, k * P:(k + 1) * P], rhs=eye,
                                     start=True, stop=True)
                    if (i + k) % 2 == 0:
                        nc.vector.tensor_copy(xT[k][:, i * P:(i + 1) * P], ptx)
                    else:
                        nc.scalar.copy(xT[k][:, i * P:(i + 1) * P], ptx)

            # ---- adj stream: row sums + on-chip transpose ------------------
            adjt = [persist.tile([P, n], f32, name=f"adjt{j}", tag=f"adjt{j}")
                    for j in range(T)]
            for i in range(T):
                at = astage.tile([P, n], f32, name="at", tag="at")
                nc.sync.dma_start(out=at, in_=adj_d[i * P:(i + 1) * P, :])
                nc.vector.reduce_sum(d_sb[:, i:i + 1], at[:, 0:DVE_COLS], axis=AX.X)
                dum = small.tile([P, n - DVE_COLS], f32, name="dum", tag="dum", bufs=1)
                nc.scalar.activation(dum, at[:, DVE_COLS:n], AF.Copy,
                                     accum_out=da_sb[:, i:i + 1])
                for j in range(T):
                    pt2 = ps.tile([P, P], f32, name="pt2", tag="ptrans", bufs=2)
                    nc.tensor.matmul(pt2, lhsT=at[:, j * P:(j + 1) * P], rhs=eye,
                                     start=True, stop=True)
                    if (i + j) % 2 == 0:
                        nc.vector.tensor_copy(adjt[j][:, i * P:(i + 1) * P], pt2)
                    else:
                        nc.scalar.copy(adjt[j][:, i * P:(i + 1) * P], pt2)

            # ---- dinv = (d + 1)**-0.5 --------------------------------------
            nc.vector.tensor_add(d2_sb, d_sb, da_sb)
            nc.scalar.activation(srt_sb, d2_sb, AF.Sqrt, bias=1.0)
            nc.vector.reciprocal(dinv, srt_sb)

            # ---- mv = relu(x@Wm + bm); y1 = dinv * mv ----------------------
            y1 = [persist.tile([P, din], f32, name=f"y1_{i}", tag=f"y1_{i}")
                  for i in range(T)]
            for i in range(T):
                pm = ps.tile([P, din], f32, name="pm", tag="pw", bufs=2)
                for k in range(KD):
                    nc.tensor.matmul(pm, lhsT=xT[k][:, i * P:(i + 1) * P],
                                     rhs=wm_sb[k], start=(k == 0), stop=(k == KD - 1))
                pre = small.tile([P, din], f32, name="pre", tag="pre")
                nc.vector.tensor_add(pre, pm, bm_sb)
                mvt = small.tile([P, din], f32, name="mvt", tag="mvt")
                nc.scalar.activation(mvt, pre, AF.Relu)
                nc.sync.dma_start(out=mv_d[i * P:(i + 1) * P, :], in_=mvt)
                nc.vector.tensor_scalar(out=y1[i], in0=pre, scalar1=dinv[:, i:i + 1],
                                        scalar2=0.0, op0=OP.mult, op1=OP.max)

            # ---- A-products: tdst[db][:, c] = (adj @ y + y).T chunks -------
            def a_product(ysrc, kb, tdst):
                for c in range(NCH):
                    c0 = c * CH
                    for db in range(kb):
                        pa = ps.tile([P, CH], f32, name="pa", tag="pa", bufs=4)
                        mms = []
                        for m in range(T):
                            mms.append((ysrc[m][:, db * P:(db + 1) * P],
                                        adjt[m][:, c0:c0 + CH], pa[:, :]))
                            if c0 <= m * P < c0 + CH:
                                off = m * P - c0
                                mms.append((ysrc[m][:, db * P:(db + 1) * P],
                                            eye, pa[:, off:off + P]))
                        for q, (l, r, o) in enumerate(mms):
                            nc.tensor.matmul(o, lhsT=l, rhs=r, start=(q == 0),
                                             stop=(q == len(mms) - 1))
                        if (c + db) % 2 == 0:
                            nc.vector.tensor_copy(tdst[db][:, c0:c0 + CH], pa)
                        else:
                            nc.scalar.copy(tdst[db][:, c0:c0 + CH], pa)

            t1T = [feat.tile([P, n], f32, name=f"t1T{k}", tag="featbig")
                   for k in range(KD)]
            a_product(y1, KD, t1T)

            # ---- hid = relu(dinv*(t1@W1) + b1); y2 = dinv * hid ------------
            y2 = [persist.tile([P, dhid], f32, name=f"y2_{i}", tag=f"y2_{i}")
                  for i in range(T)]
            for i in range(T):
                ph = ps.tile([P, dhid], f32, name="ph", tag="pw", bufs=2)
                for k in range(KD):
                    nc.tensor.matmul(ph, lhsT=t1T[k][:, i * P:(i + 1) * P],
                                     rhs=w1_sb[k], start=(k == 0), stop=(k == KD - 1))
                pre1 = small.tile([P, dhid], f32, name="pre1", tag="pre")
                nc.vector.scalar_tensor_tensor(pre1, ph, dinv[:, i:i + 1], b1_sb,
                                               op0=OP.mult, op1=OP.add)
                hidt = small.tile([P, dhid], f32, name="hidt", tag="mvt")
                nc.scalar.activation(hidt, pre1, AF.Relu)
                nc.sync.dma_start(out=hid_d[i * P:(i + 1) * P, :], in_=hidt)
                nc.vector.tensor_scalar(out=y2[i], in0=pre1, scalar1=dinv[:, i:i + 1],
                                        scalar2=0.0, op0=OP.mult, op1=OP.max)

            t2T = [feat.tile([P, n], f32, name=f"t2T{k}", tag="featbig")
                   for k in range(KH)]
            a_product(y2, KH, t2T)

            # ---- out = relu(dinv*(t2@W2) + b2) -----------------------------
            for i in range(T):
                po = ps.tile([P, dout], f32, name="po", tag="pw", bufs=2)
                for k in range(KH):
                    nc.tensor.matmul(po, lhsT=t2T[k][:, i * P:(i + 1) * P],
                                     rhs=w2_sb[k], start=(k == 0), stop=(k == KH - 1))
                pre2 = small.tile([P, dout], f32, name="pre2", tag="pre")
                nc.vector.scalar_tensor_tensor(pre2, po, dinv[:, i:i + 1], b2_sb,
                                               op0=OP.mult, op1=OP.add)
                outt = small.tile([P, dout], f32, name="outt", tag="mvt")
                nc.scalar.activation(outt, pre2, AF.Relu)
                nc.sync.dma_start(out=out_d[i * P:(i + 1) * P, :], in_=outt)

    nc.compile()
    return nc


_NC_CACHE = None


def _get_nc():
    global _NC_CACHE
    if _NC_CACHE is None:
        _NC_CACHE = _build_module()
    return _NC_CACHE


def _make_in_maps(adj, x, w_mean, b_mean, w1, b1, w2, b2):
    adj = np.asarray(adj, dtype=np.float32)
    x = np.asarray(x, dtype=np.float32)
    wm = np.ascontiguousarray(np.asarray(w_mean, dtype=np.float32))
    w1 = np.ascontiguousarray(np.asarray(w1, dtype=np.float32))
    w2 = np.ascontiguousarray(np.asarray(w2, dtype=np.float32))
    bm = np.ascontiguousarray(np.broadcast_to(np.asarray(b_mean, np.float32), (P, IN_DIM)))
    b1b = np.ascontiguousarray(np.broadcast_to(np.asarray(b1, np.float32), (P, HID_DIM)))
    b2b = np.ascontiguousarray(np.broadcast_to(np.asarray(b2, np.float32), (P, OUT_DIM)))
    eye = np.eye(P, dtype=np.float32)
    return [
        dict(adj=np.ascontiguousarray(adj[b]), x=np.ascontiguousarray(x[b]),
             wm=wm, w1=w1, w2=w2, bm=bm, b1=b1b, b2=b2b, eye=eye)
        for b in range(B)
    ]


def kernel(adj, gcn_inputs, w_mean, b_mean, w1, b1, w2, b2):
    from concourse.bass_utils import run_bass_kernel_spmd

    nc = _get_nc()
    in_maps = _make_in_maps(adj, gcn_inputs, w_mean, b_mean, w1, b1, w2, b2)
    res = run_bass_kernel_spmd(nc, in_maps, core_ids=list(range(B)))
    mv = np.stack([res.results[b]["mv"] for b in range(B)])
    hid = np.stack([res.results[b]["hid"] for b in range(B)])
    out = np.stack([res.results[b]["out"] for b in range(B)])
    x = np.asarray(gcn_inputs, dtype=np.float32)
    return ((x, mv, hid, out), ())


# revision 3
# speedup vs baseline: 1.9028x; 1.9028x over previous
"""GCN (CGCN) forward kernel for Trainium2, data-parallel over batch on 8 NeuronCores.

Per core (one batch sample):
  d      = adj.sum(-1) + 1 ;  dinv = d**-0.5
  mv     = relu(x @ Wm + bm)
  t1     = adj @ (dinv*mv) + (dinv*mv)          # A @ Dinv @ mv  with A = adj + I
  hid    = relu(dinv*(t1 @ W1) + b1)
  t2     = adj @ (dinv*hid) + (dinv*hid)
  out    = relu(dinv*(t2 @ W2) + b2)

adj is streamed from HBM exactly once; its transpose (needed because the PE
contracts over the partition axis) is built on-chip with identity matmuls and
kept resident in SBUF.
"""

import numpy as np

B, N, IN_DIM, HID_DIM, OUT_DIM = 8, 2048, 256, 256, 128
P = 128


def _build_module(n=N, din=IN_DIM, dhid=HID_DIM, dout=OUT_DIM, astage_bufs=3):
    from contextlib import ExitStack

    import concourse.mybir as mybir
    import concourse.tile as tile
    from concourse import bacc

    f32 = mybir.dt.float32
    AX = mybir.AxisListType
    OP = mybir.AluOpType
    AF = mybir.ActivationFunctionType

    T = n // P            # node tiles
    KD = din // P         # input-feature blocks
    KH = dhid // P        # hidden-feature blocks
    CH = min(512, n)      # A-product chunk width (one PSUM bank)
    NCH = n // CH
    DVE_COLS = n // 2     # row-sum split between VectorE and ScalarE

    nc = bacc.Bacc(None, target_bir_lowering=False, dynamic_dma_scratch_size=2048)

    adj_d = nc.declare_dram_parameter("adj", [n, n], f32, isOutput=False)
    x_d = nc.declare_dram_parameter("x", [n, din], f32, isOutput=False)
    wm_d = nc.declare_dram_parameter("wm", [din, din], f32, isOutput=False)
    w1_d = nc.declare_dram_parameter("w1", [din, dhid], f32, isOutput=False)
    w2_d = nc.declare_dram_parameter("w2", [dhid, dout], f32, isOutput=False)
    bm_d = nc.declare_dram_parameter("bm", [P, din], f32, isOutput=False)
    b1_d = nc.declare_dram_parameter("b1", [P, dhid], f32, isOutput=False)
    b2_d = nc.declare_dram_parameter("b2", [P, dout], f32, isOutput=False)
    eye_d = nc.declare_dram_parameter("eye", [P, P], f32, isOutput=False)
    mv_d = nc.declare_dram_parameter("mv", [n, din], f32, isOutput=True)
    hid_d = nc.declare_dram_parameter("hid", [n, dhid], f32, isOutput=True)
    out_d = nc.declare_dram_parameter("out", [n, dout], f32, isOutput=True)

    with tile.TileContext(nc) as tc:
        with ExitStack() as ctx:
            persist = ctx.enter_context(tc.tile_pool(name="persist", bufs=1))
            astage = ctx.enter_context(tc.tile_pool(name="astage", bufs=astage_bufs))
            small = ctx.enter_context(tc.tile_pool(name="small", bufs=3))
            feat = ctx.enter_context(tc.tile_pool(name="feat", bufs=2))
            ps = ctx.enter_context(tc.tile_pool(name="ps", bufs=2, space="PSUM"))

            # ---- constants -------------------------------------------------
            eye = persist.tile([P, P], f32, name="eye", tag="eye")
            nc.sync.dma_start(out=eye, in_=eye_d[:, :])
            wm_sb, w1_sb, w2_sb = [], [], []
            for k in range(KD):
                t = persist.tile([P, din], f32, name=f"wm{k}", tag=f"wm{k}")
                nc.sync.dma_start(out=t, in_=wm_d[k * P:(k + 1) * P, :])
                wm_sb.append(t)
            for k in range(KD):
                t = persist.tile([P, dhid], f32, name=f"w1_{k}", tag=f"w1_{k}")
                nc.sync.dma_start(out=t, in_=w1_d[k * P:(k + 1) * P, :])
                w1_sb.append(t)
            for k in range(KH):
                t = persist.tile([P, dout], f32, name=f"w2_{k}", tag=f"w2_{k}")
                nc.sync.dma_start(out=t, in_=w2_d[k * P:(k + 1) * P, :])
                w2_sb.append(t)
            bm_sb = persist.tile([P, din], f32, name="bm_sb", tag="bm_sb")
            nc.sync.dma_start(out=bm_sb, in_=bm_d[:, :])
            b1_sb = persist.tile([P, dhid], f32, name="b1_sb", tag="b1_sb")
            nc.sync.dma_start(out=b1_sb, in_=b1_d[:, :])
            b2_sb = persist.tile([P, dout], f32, name="b2_sb", tag="b2_sb")
            nc.sync.dma_start(out=b2_sb, in_=b2_d[:, :])

            d_sb = persist.tile([P, T], f32, name="d_sb", tag="d_sb")
            da_sb = persist.tile([P, T], f32, name="da_sb", tag="da_sb")
            d2_sb = persist.tile([P, T], f32, name="d2_sb", tag="d2_sb")
            srt_sb = persist.tile([P, T], f32, name="srt_sb", tag="srt_sb")
            dinv = persist.tile([P, T], f32, name="dinv", tag="dinv")

            # ---- x transpose (xT[k][:, i*P:(i+1)*P] = x[iP:(i+1)P, kP:(k+1)P].T)
            xT = [feat.tile([P, n], f32, name=f"xT{k}", tag="featbig") for k in range(KD)]
            for i in range(T):
                xt = small.tile([P, din], f32, name="xt", tag="xt")
                nc.sync.dma_start(out=xt, in_=x_d[i * P:(i + 1) * P, :])
                for k in range(KD):
                    ptx = ps.tile([P, P], f32, name="ptx", tag="ptrans", bufs=2)
                    nc.tensor.matmul(ptx, lhsT=xt[:, k * P:(k + 1) * P], rhs=eye,
                                     start=True, stop=True)
                    if (i + k) % 2 == 0:
                        nc.vector.tensor_copy(xT[k][:, i * P:(i + 1) * P], ptx)
                    else:
                        nc.scalar.copy(xT[k][:, i * P:(i + 1) * P], ptx)

            # ---- adj stream: row sums + on-chip transpose ------------------
            adjt = [persist.tile([P, n], f32, name=f"adjt{j}", tag=f"adjt{j}")
                    for j in range(T)]
            for i in range(T):
                at = astage.tile([P, n], f32, name="at", tag="at")
                nc.sync.dma_start(out=at, in_=adj_d[i * P:(i + 1) * P, :])
                nc.vector.reduce_sum(d_sb[:, i:i + 1], at[:, 0:DVE_COLS], axis=AX.X)
                dum = small.tile([P, n - DVE_COLS], f32, name="dum", tag="dum", bufs=1)
                nc.scalar.activation(dum, at[:, DVE_COLS:n], AF.Copy,
                                     accum_out=da_sb[:, i:i + 1])
                for j in range(T):
                    pt2 = ps.tile([P, P], f32, name="pt2", tag="ptrans", bufs=2)
                    nc.tensor.matmul(pt2, lhsT=at[:, j * P:(j + 1) * P], rhs=eye,
                                     start=True, stop=True)
                    if (i + j) % 2 == 0:
                        nc.vector.tensor_copy(adjt[j][:, i * P:(i + 1) * P], pt2)
                    else:
                        nc.scalar.copy(adjt[j][:, i * P:(i + 1) * P], pt2)

            # ---- dinv = (d + 1)**-0.5 --------------------------------------
            nc.vector.tensor_add(d2_sb, d_sb, da_sb)
            nc.scalar.activation(srt_sb, d2_sb, AF.Sqrt, bias=1.0)
            nc.vector.reciprocal(dinv, srt_sb)

            # ---- mv = relu(x@Wm + bm); y1 = dinv * mv ----------------------
            y1 = [persist.tile([P, din], f32, name=f"y1_{i}", tag=f"y1_{i}")
                  for i in range(T)]
            for i in range(T):
                pm = ps.tile([P, din], f32, name="pm", tag="pw", bufs=2)
                for k in range(KD):
                    nc.tensor.matmul(pm, lhsT=xT[k][:, i * P:(i + 1) * P],
                                     rhs=wm_sb[k], start=(k == 0), stop=(k == KD - 1))
                pre = small.tile([P, din], f32, name="pre", tag="pre")
                nc.vector.tensor_add(pre, pm, bm_sb)
                mvt = small.tile([P, din], f32, name="mvt", tag="mvt")
                nc.scalar.activation(mvt, pre, AF.Relu)
                nc.sync.dma_start(out=mv_d[i * P:(i + 1) * P, :], in_=mvt)
                nc.vector.tensor_scalar(out=y1[i], in0=pre, scalar1=dinv[:, i:i + 1],
                                        scalar2=0.0, op0=OP.mult, op1=OP.max)

            # ---- A-products: tdst[db][:, c] = (adj @ y + y).T chunks -------
            def a_product(ysrc, kb, tdst):
                for c in range(NCH):
                    c0 = c * CH
                    for db in range(kb):
                        pa = ps.tile([P, CH], f32, name="pa", tag="pa", bufs=4)
                        mms = []
                        for m in range(T):
                            mms.append((ysrc[m][:, db * P:(db + 1) * P],
                                        adjt[m][:, c0:c0 + CH], pa[:, :]))
                            if c0 <= m * P < c0 + CH:
                                off = m * P - c0
                                mms.append((ysrc[m][:, db * P:(db + 1) * P],
                                            eye, pa[:, off:off + P]))
                        for q, (l, r, o) in enumerate(mms):
                            nc.tensor.matmul(o, lhsT=l, rhs=r, start=(q == 0),
                                             stop=(q == len(mms) - 1))
                        if (c + db) % 2 == 0:
                            nc.vector.tensor_copy(tdst[db][:, c0:c0 + CH], pa)
                        else:
                            nc.scalar.copy(tdst[db][:, c0:c0 + CH], pa)

            t1T = [feat.tile([P, n], f32, name=f"t1T{k}", tag="featbig")
                   for k in range(KD)]
            a_product(y1, KD, t1T)

            # ---- hid = relu(dinv*(t1@W1) + b1); y2 = dinv * hid ------------
            y2 = [persist.tile([P, dhid], f32, name=f"y2_{i}", tag=f"y2_{i}")
                  for i in range(T)]
            for i in range(T):
                ph = ps.tile([P, dhid], f32, name="ph", tag="pw", bufs=2)
                for k in range(KD):
                    nc.tensor.matmul(ph, lhsT=t1T[k][:, i * P:(i + 1) * P],
                                     rhs=w1_sb[k], start=(k == 0), stop=(k == KD - 1))
                pre1 = small.tile([P, dhid], f32, name="pre1", tag="pre")
                nc.vector.scalar_tensor_tensor(pre1, ph, dinv[:, i:i + 1], b1_sb,
                                               op0=OP.mult, op1=OP.add)
                hidt = small.tile([P, dhid], f32, name="hidt", tag="mvt")
                nc.scalar.activation(hidt, pre1, AF.Relu)
                nc.sync.dma_start(out=hid_d[i * P:(i + 1) * P, :], in_=hidt)
                nc.vector.tensor_scalar(out=y2[i], in0=pre1, scalar1=dinv[:, i:i + 1],
                                        scalar2=0.0, op0=OP.mult, op1=OP.max)

            t2T = [feat.tile([P, n], f32, name=f"t2T{k}", tag="featbig")
                   for k in range(KH)]
            a_product(y2, KH, t2T)

            # ---- out = relu(dinv*(t2@W2) + b2) -----------------------------
            for i in range(T):
                po = ps.tile([P, dout], f32, name="po", tag="pw", bufs=2)
                for k in range(KH):
                    nc.tensor.matmul(po, lhsT=t2T[k][:, i * P:(i + 1) * P],
                                     rhs=w2_sb[k], start=(k == 0), stop=(k == KH - 1))
                pre2 = small.tile([P, dout], f32, name="pre2", tag="pre")
                nc.vector.scalar_tensor_tensor(pre2, po, dinv[:, i:i + 1], b2_sb,
                                               op0=OP.mult, op1=OP.add)
                outt = small.tile([P, dout], f32, name="outt", tag="mvt")
                nc.scalar.activation(outt, pre2, AF.Relu)
                nc.sync.dma_start(out=out_d[i * P:(i + 1) * P, :], in_=outt)

    nc.compile()
    return nc


def _build_module_bf16(n=N, din=IN_DIM, dhid=HID_DIM, dout=OUT_DIM):
    """v3: adj in bf16. adj is transposed by the DMA xbar engine straight from
    DRAM into a resident SBUF tensor; row sums come from PE ones-products on the
    transposed tiles (no natural-layout adj load at all). The W-product chain
    stays fp32."""
    from contextlib import ExitStack

    import concourse.mybir as mybir
    import concourse.tile as tile
    from concourse import bacc

    f32 = mybir.dt.float32
    bf16 = mybir.dt.bfloat16
    AX = mybir.AxisListType
    OP = mybir.AluOpType
    AF = mybir.ActivationFunctionType

    T = n // P
    KD = din // P
    KH = dhid // P
    CH = min(512, n)
    NCH = n // CH

    nc = bacc.Bacc(None, target_bir_lowering=False, dynamic_dma_scratch_size=2048)

    adjb_d = nc.declare_dram_parameter("adjb", [n, n], bf16, isOutput=False)
    x_d = nc.declare_dram_parameter("x", [n, din], f32, isOutput=False)
    wm_d = nc.declare_dram_parameter("wm", [din, din], f32, isOutput=False)
    w1_d = nc.declare_dram_parameter("w1", [din, dhid], f32, isOutput=False)
    w2_d = nc.declare_dram_parameter("w2", [dhid, dout], f32, isOutput=False)
    bm_d = nc.declare_dram_parameter("bm", [P, din], f32, isOutput=False)
    b1_d = nc.declare_dram_parameter("b1", [P, dhid], f32, isOutput=False)
    b2_d = nc.declare_dram_parameter("b2", [P, dout], f32, isOutput=False)
    eye_d = nc.declare_dram_parameter("eye", [P, P], f32, isOutput=False)
    eyeb_d = nc.declare_dram_parameter("eyeb", [P, P], bf16, isOutput=False)
    onesb_d = nc.declare_dram_parameter("onesb", [P, 1], bf16, isOutput=False)
    onef_d = nc.declare_dram_parameter("onef", [1, 1], f32, isOutput=False)
    mv_d = nc.declare_dram_parameter("mv", [n, din], f32, isOutput=True)
    hid_d = nc.declare_dram_parameter("hid", [n, dhid], f32, isOutput=True)
    out_d = nc.declare_dram_parameter("out", [n, dout], f32, isOutput=True)

    with tile.TileContext(nc) as tc:
        with ExitStack() as ctx:
            persist = ctx.enter_context(tc.tile_pool(name="persist", bufs=1))
            small = ctx.enter_context(tc.tile_pool(name="small", bufs=3))
            feat = ctx.enter_context(tc.tile_pool(name="feat", bufs=2))
            ps = ctx.enter_context(tc.tile_pool(name="ps", bufs=2, space="PSUM"))

            # ---- constants -------------------------------------------------
            eye = persist.tile([P, P], f32, name="eye", tag="eye")
            nc.sync.dma_start(out=eye, in_=eye_d[:, :])
            eyeb = persist.tile([P, P], bf16, name="eyeb", tag="eyeb")
            nc.sync.dma_start(out=eyeb, in_=eyeb_d[:, :])
            onesb = persist.tile([P, 1], bf16, name="onesb", tag="onesb")
            nc.sync.dma_start(out=onesb, in_=onesb_d[:, :])
            onef = persist.tile([1, 1], f32, name="onef", tag="onef")
            nc.sync.dma_start(out=onef, in_=onef_d[:, :])
            wm_sb, w1_sb, w2_sb = [], [], []
            for k in range(KD):
                t = persist.tile([P, din], f32, name=f"wm{k}", tag=f"wm{k}")
                nc.sync.dma_start(out=t, in_=wm_d[k * P:(k + 1) * P, :])
                wm_sb.append(t)
            for k in range(KD):
                t = persist.tile([P, dhid], f32, name=f"w1_{k}", tag=f"w1_{k}")
                nc.sync.dma_start(out=t, in_=w1_d[k * P:(k + 1) * P, :])
                w1_sb.append(t)
            for k in range(KH):
                t = persist.tile([P, dout], f32, name=f"w2_{k}", tag=f"w2_{k}")
                nc.sync.dma_start(out=t, in_=w2_d[k * P:(k + 1) * P, :])
                w2_sb.append(t)
            bm_sb = persist.tile([P, din], f32, name="bm_sb", tag="bm_sb")
            nc.sync.dma_start(out=bm_sb, in_=bm_d[:, :])
            b1_sb = persist.tile([P, dhid], f32, name="b1_sb", tag="b1_sb")
            nc.sync.dma_start(out=b1_sb, in_=b1_d[:, :])
            b2_sb = persist.tile([P, dout], f32, name="b2_sb", tag="b2_sb")
            nc.sync.dma_start(out=b2_sb, in_=b2_d[:, :])

            dinv = persist.tile([P, T], f32, name="dinv", tag="dinv")

            # adjT resident in SBUF: adjt[mp, j, nf] = adj[nf_global, j*128+mp]
            adjt = persist.tile([P, T, n], bf16, name="adjt", tag="adjt")

            xT = [feat.tile([P, n], f32, name=f"xT{k}", tag="featbig")
                  for k in range(KD)]
            y1 = [persist.tile([P, din], bf16, name=f"y1_{i}", tag=f"y1_{i}")
                  for i in range(T)]

            # ---- stream: transpose-load adj columns, per-tile d/dinv, x, Wm
            for i in range(T):
                isl = slice(i * P, (i + 1) * P)
                nc.sync.dma_start_transpose(adjt[:, :, isl], adjb_d[isl, :])

                # d for output rows of tile i: sum over all m of adjT[:, :, i]
                pd = ps.tile([1, P], f32, name="pd", tag="pdrow", bufs=2)
                for j in range(T):
                    nc.tensor.matmul(pd, lhsT=onesb, rhs=adjt[:, j, isl],
                                     start=(j == 0), stop=(j == T - 1))
                drow = small.tile([1, P], f32, name="drow", tag="drow")
                nc.vector.tensor_copy(drow, pd)
                pcol = ps.tile([P, 1], f32, name="pcol", tag="pdrow", bufs=2)
                nc.tensor.matmul(pcol, lhsT=drow, rhs=onef, start=True, stop=True)
                srt = small.tile([P, 1], f32, name="srt", tag="srt")
                nc.scalar.activation(srt, pcol, AF.Sqrt, bias=1.0)
                nc.vector.reciprocal(dinv[:, i:i + 1], srt)

                # x tile -> xT (fp32 PE transpose)
                xt = small.tile([P, din], f32, name="xt", tag="xt")
                nc.sync.dma_start(out=xt, in_=x_d[isl, :])
                for k in range(KD):
                    ptx = ps.tile([P, P], f32, name="ptx", tag="ptmp", bufs=4)
                    nc.tensor.matmul(ptx, lhsT=xt[:, k * P:(k + 1) * P], rhs=eye,
                                     start=True, stop=True)
                    if (i + k) % 2 == 0:
                        nc.vector.tensor_copy(xT[k][:, isl], ptx)
                    else:
                        nc.scalar.copy(xT[k][:, isl], ptx)

                # mv = relu(x@Wm + bm); y1 = bf16(dinv * mv)
                pm = ps.tile([P, din], f32, name="pm", tag="pw", bufs=2)
                for k in range(KD):
                    nc.tensor.matmul(pm, lhsT=xT[k][:, isl], rhs=wm_sb[k],
                                     start=(k == 0), stop=(k == KD - 1))
                pre = small.tile([P, din], f32, name="pre", tag="pre")
                nc.vector.tensor_add(pre, pm, bm_sb)
                mvt = small.tile([P, din], f32, name="mvt", tag="mvt")
                nc.scalar.activation(mvt, pre, AF.Relu)
                nc.sync.dma_start(out=mv_d[isl, :], in_=mvt)
                nc.vector.tensor_scalar(out=y1[i], in0=pre, scalar1=dinv[:, i:i + 1],
                                        scalar2=0.0, op0=OP.mult, op1=OP.max)

            # ---- A-products (bf16): tdst[db][:, c] = (adj @ y + y).T ------
            def a_product(ysrc, kb, tdst):
                for c in range(NCH):
                    c0 = c * CH
                    for db in range(kb):
                        pa = ps.tile([P, CH], f32, name="pa", tag="ptmp", bufs=4)
                        mms = []
                        for m in range(T):
                            mms.append((ysrc[m][:, db * P:(db + 1) * P],
                                        adjt[:, m, c0:c0 + CH], pa[:, :]))
                            if c0 <= m * P < c0 + CH:
                                off = m * P - c0
                                mms.append((ysrc[m][:, db * P:(db + 1) * P],
                                            eyeb, pa[:, off:off + P]))
                        for q, (l, r, o) in enumerate(mms):
                            nc.tensor.matmul(o, lhsT=l, rhs=r, start=(q == 0),
                                             stop=(q == len(mms) - 1))
                        if (c + db) % 2 == 0:
                            nc.vector.tensor_copy(tdst[db][:, c0:c0 + CH], pa)
                        else:
                            nc.scalar.copy(tdst[db][:, c0:c0 + CH], pa)

            t1T = [feat.tile([P, n], f32, name=f"t1T{k}", tag="featbig")
                   for k in range(KD)]
            a_product(y1, KD, t1T)

            # ---- hid = relu(dinv*(t1@W1) + b1); y2 = bf16(dinv * hid) ------
            y2 = [persist.tile([P, dhid], bf16, name=f"y2_{i}", tag=f"y2_{i}")
                  for i in range(T)]
            for i in range(T):
                isl = slice(i * P, (i + 1) * P)
                ph = ps.tile([P, dhid], f32, name="ph", tag="pw", bufs=2)
                for k in range(KD):
                    nc.tensor.matmul(ph, lhsT=t1T[k][:, isl], rhs=w1_sb[k],
                                     start=(k == 0), stop=(k == KD - 1))
                pre1 = small.tile([P, dhid], f32, name="pre1", tag="pre")
                nc.vector.scalar_tensor_tensor(pre1, ph, dinv[:, i:i + 1], b1_sb,
                                               op0=OP.mult, op1=OP.add)
                hidt = small.tile([P, dhid], f32, name="hidt", tag="mvt")
                nc.scalar.activation(hidt, pre1, AF.Relu)
                nc.sync.dma_start(out=hid_d[isl, :], in_=hidt)
                nc.vector.tensor_scalar(out=y2[i], in0=pre1, scalar1=dinv[:, i:i + 1],
                                        scalar2=0.0, op0=OP.mult, op1=OP.max)

            t2T = [feat.tile([P, n], f32, name=f"t2T{k}", tag="featbig")
                   for k in range(KH)]
            a_product(y2, KH, t2T)

            # ---- out = relu(dinv*(t2@W2) + b2) -----------------------------
            for i in range(T):
                isl = slice(i * P, (i + 1) * P)
                po = ps.tile([P, dout], f32, name="po", tag="pw", bufs=2)
                for k in range(KH):
                    nc.tensor.matmul(po, lhsT=t2T[k][:, isl], rhs=w2_sb[k],
                                     start=(k == 0), stop=(k == KH - 1))
                pre2 = small.tile([P, dout], f32, name="pre2", tag="pre")
                nc.vector.scalar_tensor_tensor(pre2, po, dinv[:, i:i + 1], b2_sb,
                                               op0=OP.mult, op1=OP.add)
                outt = small.tile([P, dout], f32, name="outt", tag="mvt")
                nc.scalar.activation(outt, pre2, AF.Relu)
                nc.sync.dma_start(out=out_d[isl, :], in_=outt)

    nc.compile()
    return nc


VARIANT = "bf16"
_NC_CACHE = None


def _get_nc():
    global _NC_CACHE
    if _NC_CACHE is None:
        if VARIANT == "bf16":
            _NC_CACHE = _build_module_bf16()
        else:
            _NC_CACHE = _build_module()
    return _NC_CACHE


def _make_in_maps(adj, x, w_mean, b_mean, w1, b1, w2, b2):
    import ml_dtypes

    adj = np.asarray(adj, dtype=np.float32)
    x = np.asarray(x, dtype=np.float32)
    wm = np.ascontiguousarray(np.asarray(w_mean, dtype=np.float32))
    w1 = np.ascontiguousarray(np.asarray(w1, dtype=np.float32))
    w2 = np.ascontiguousarray(np.asarray(w2, dtype=np.float32))
    bm = np.ascontiguousarray(np.broadcast_to(np.asarray(b_mean, np.float32), (P, IN_DIM)))
    b1b = np.ascontiguousarray(np.broadcast_to(np.asarray(b1, np.float32), (P, HID_DIM)))
    b2b = np.ascontiguousarray(np.broadcast_to(np.asarray(b2, np.float32), (P, OUT_DIM)))
    eye = np.eye(P, dtype=np.float32)
    if VARIANT == "bf16":
        eyeb = np.eye(P, dtype=ml_dtypes.bfloat16)
        onesb = np.ones((P, 1), dtype=ml_dtypes.bfloat16)
        onef = np.ones((1, 1), dtype=np.float32)
        return [
            dict(adjb=np.ascontiguousarray(adj[b].astype(ml_dtypes.bfloat16)),
                 x=np.ascontiguousarray(x[b]),
                 wm=wm, w1=w1, w2=w2, bm=bm, b1=b1b, b2=b2b,
                 eye=eye, eyeb=eyeb, onesb=onesb, onef=onef)
            for b in range(B)
        ]
    return [
        dict(adj=np.ascontiguousarray(adj[b]), x=np.ascontiguousarray(x[b]),
             wm=wm, w1=w1, w2=w2, bm=bm, b1=b1b, b2=b2b, eye=eye)
        for b in range(B)
    ]


def kernel(adj, gcn_inputs, w_mean, b_mean, w1, b1, w2, b2):
    from concourse.bass_utils import run_bass_kernel_spmd

    nc = _get_nc()
    in_maps = _make_in_maps(adj, gcn_inputs, w_mean, b_mean, w1, b1, w2, b2)
    res = run_bass_kernel_spmd(nc, in_maps, core_ids=list(range(B)))
    mv = np.stack([res.results[b]["mv"] for b in range(B)])
    hid = np.stack([res.results[b]["hid"] for b in range(B)])
    out = np.stack([res.results[b]["out"] for b in range(B)])
    x = np.asarray(gcn_inputs, dtype=np.float32)
    return ((x, mv, hid, out), ())


# revision 17
# speedup vs baseline: 2.1502x; 1.1300x over previous
"""GCN (CGCN) forward kernel for Trainium2, data-parallel over batch on 8 NeuronCores.

Per core (one batch sample):
  d      = adj.sum(-1) + 1 ;  dinv = d**-0.5
  mv     = relu(x @ Wm + bm)
  t1     = adj @ (dinv*mv) + (dinv*mv)          # A @ Dinv @ mv  with A = adj + I
  hid    = relu(dinv*(t1 @ W1) + b1)
  t2     = adj @ (dinv*hid) + (dinv*hid)
  out    = relu(dinv*(t2 @ W2) + b2)

adj is shipped to the device in bf16 and transposed by the DMA xbar engine
straight from DRAM into a resident SBUF tensor (the PE contracts over the
partition axis, so adj@v needs adj^T tiles).  Row sums come from PE
ones-products over the transposed tiles, so adj is read from HBM exactly once.
All matmuls run in bf16 with fp32 PSUM accumulation; everything else
(normalization, biases, relu, outputs) stays fp32.
"""

import numpy as np

B, N, IN_DIM, HID_DIM, OUT_DIM = 8, 2048, 256, 256, 128
P = 128


def _build_module_bf16(n=N, din=IN_DIM, dhid=HID_DIM, dout=OUT_DIM):
    from contextlib import ExitStack

    import concourse.mybir as mybir
    import concourse.tile as tile
    from concourse import bacc

    f32 = mybir.dt.float32
    f32r = mybir.dt.float32r
    bf16 = mybir.dt.bfloat16
    OP = mybir.AluOpType
    AF = mybir.ActivationFunctionType

    T = n // P
    KD = din // P
    KH = dhid // P
    CH = min(512, n)
    NCH = n // CH
    TC_ = CH // P  # stream tiles per chunk

    nc = bacc.Bacc(None, target_bir_lowering=False)

    adjb_d = nc.declare_dram_parameter("adjb", [n, n], bf16, isOutput=False)
    x_d = nc.declare_dram_parameter("x", [n, din], f32, isOutput=False)
    wm_d = nc.declare_dram_parameter("wm", [din, din], f32, isOutput=False)
    w1_d = nc.declare_dram_parameter("w1", [din, dhid], f32, isOutput=False)
    w2_d = nc.declare_dram_parameter("w2", [dhid, dout], f32, isOutput=False)
    bm_d = nc.declare_dram_parameter("bm", [P, din], f32, isOutput=False)
    b1_d = nc.declare_dram_parameter("b1", [P, dhid], f32, isOutput=False)
    b2_d = nc.declare_dram_parameter("b2", [P, dout], f32, isOutput=False)
    eyeb_d = nc.declare_dram_parameter("eyeb", [P, P], bf16, isOutput=False)
    eyef_d = nc.declare_dram_parameter("eyef", [P, P], f32, isOutput=False)
    onesb_d = nc.declare_dram_parameter("onesb", [P, 1], bf16, isOutput=False)
    onef_d = nc.declare_dram_parameter("onef", [1, 1], f32, isOutput=False)
    mv_d = nc.declare_dram_parameter("mv", [n, din], f32, isOutput=True)
    hid_d = nc.declare_dram_parameter("hid", [n, dhid], f32, isOutput=True)
    out_d = nc.declare_dram_parameter("out", [n, dout], f32, isOutput=True)

    with tile.TileContext(nc) as tc:
        with ExitStack() as ctx:
            persist = ctx.enter_context(tc.tile_pool(name="persist", bufs=1))
            small = ctx.enter_context(tc.tile_pool(name="small", bufs=4))
            feat = ctx.enter_context(tc.tile_pool(name="feat", bufs=2))
            ps = ctx.enter_context(tc.tile_pool(name="ps", bufs=2, space="PSUM"))

            # ---- constants (gpsimd ring; weights cast to bf16 on the DMA) --
            eyeb = persist.tile([P, P], bf16, name="eyeb", tag="eyeb")
            nc.gpsimd.dma_start(out=eyeb, in_=eyeb_d[:, :])
            eyer = persist.tile([P, P], f32r, name="eyer", tag="eyer")
            nc.gpsimd.dma_start(out=eyer, in_=eyef_d[:, :])
            onesb = persist.tile([P, 1], bf16, name="onesb", tag="onesb")
            nc.gpsimd.dma_start(out=onesb, in_=onesb_d[:, :])
            onef = persist.tile([1, 1], f32, name="onef", tag="onef")
            nc.gpsimd.dma_start(out=onef, in_=onef_d[:, :])
            wm_sb, w1_sb, w2_sb = [], [], []
            for k in range(KD):
                t = persist.tile([P, din], f32r, name=f"wm{k}", tag=f"wm{k}")
                nc.gpsimd.dma_start(out=t, in_=wm_d[k * P:(k + 1) * P, :])
                wm_sb.append(t)
            for k in range(KD):
                t = persist.tile([P, dhid], f32r, name=f"w1_{k}", tag=f"w1_{k}")
                nc.gpsimd.dma_start(out=t, in_=w1_d[k * P:(k + 1) * P, :])
                w1_sb.append(t)
            for k in range(KH):
                t = persist.tile([P, dout], f32r, name=f"w2_{k}", tag=f"w2_{k}")
                nc.gpsimd.dma_start(out=t, in_=w2_d[k * P:(k + 1) * P, :])
                w2_sb.append(t)
            bm_sb = persist.tile([P, din], f32, name="bm_sb", tag="bm_sb")
            nc.gpsimd.dma_start(out=bm_sb, in_=bm_d[:, :])
            b1_sb = persist.tile([P, dhid], f32, name="b1_sb", tag="b1_sb")
            nc.gpsimd.dma_start(out=b1_sb, in_=b1_d[:, :])
            b2_sb = persist.tile([P, dout], f32, name="b2_sb", tag="b2_sb")
            nc.gpsimd.dma_start(out=b2_sb, in_=b2_d[:, :])

            dinv = persist.tile([P, T], f32, name="dinv", tag="dinv")

            # adjT resident in SBUF: adjt[mp, j, nf] = adj[nf_global, j*128+mp]
            adjt = persist.tile([P, T, n], bf16, name="adjt", tag="adjt")

            xT = [persist.tile([P, n], f32r, name=f"xT{k}", tag=f"xT{k}")
                  for k in range(KD)]
            y1 = [persist.tile([P, din], bf16, name=f"y1_{i}", tag=f"y1_{i}")
                  for i in range(T)]

            # ---- stream: per chunk of 4 tiles: transpose-loads, d, x, Wm ---
            for c in range(NCH):
                tiles = range(c * TC_, (c + 1) * TC_)
                xb = {}
                for i in tiles:
                    isl = slice(i * P, (i + 1) * P)
                    # adj columns i arrive transposed (SP ring, nothing else on it)
                    nc.sync.dma_start_transpose(adjt[:, :, isl], adjb_d[isl, :])
                    # x tile, rounded to f32r during the SWDGE transfer
                    xb[i] = small.tile([P, din], f32r, name="xbt", tag="xbt", bufs=8)
                    nc.gpsimd.dma_start(out=xb[i], in_=x_d[isl, :])

                # d for output rows of this chunk: ones^T-product over adjT
                csl = slice(c * CH, (c + 1) * CH)
                pdr = ps.tile([1, CH], f32, name="pdr", tag="pdrow", bufs=2)
                for j in range(T):
                    nc.tensor.matmul(pdr, lhsT=onesb, rhs=adjt[:, j, csl],
                                     start=(j == 0), stop=(j == T - 1))
                drow = small.tile([1, CH], f32, name="drow", tag="drow")
                nc.vector.tensor_copy(drow, pdr)
                for q, i in enumerate(tiles):
                    pcol = ps.tile([P, 1], f32, name="pcol", tag="pdrow", bufs=2)
                    nc.tensor.matmul(pcol, lhsT=drow[0:1, q * P:(q + 1) * P],
                                     rhs=onef, start=True, stop=True)
                    srt = small.tile([P, 1], f32, name="srt", tag="srt")
                    nc.scalar.activation(srt, pcol, AF.Sqrt, bias=1.0)
                    nc.vector.reciprocal(dinv[:, i:i + 1], srt)

                for i in tiles:
                    isl = slice(i * P, (i + 1) * P)
                    # xT via PE transpose (f32r identity matmul)
                    for k in range(KD):
                        ptx = ps.tile([P, P], f32, name="ptx", tag="ptmp", bufs=4)
                        nc.tensor.matmul(ptx, lhsT=xb[i][:, k * P:(k + 1) * P],
                                         rhs=eyer, start=True, stop=True)
                        if (i + k) % 2 == 0:
                            nc.vector.tensor_copy(xT[k][:, isl], ptx)
                        else:
                            nc.scalar.copy(xT[k][:, isl], ptx)
                    # mv = relu(x@Wm + bm); y1 = bf16(dinv * mv)
                    pm = ps.tile([P, din], f32, name="pm", tag="pw", bufs=2)
                    for k in range(KD):
                        nc.tensor.matmul(pm, lhsT=xT[k][:, isl], rhs=wm_sb[k],
                                         start=(k == 0), stop=(k == KD - 1))
                    pre = small.tile([P, din], f32, name="pre", tag="pre")
                    nc.vector.tensor_add(pre, pm, bm_sb)
                    mvt = small.tile([P, din], f32, name="mvt", tag="mvt")
                    nc.scalar.activation(mvt, pre, AF.Relu)
                    nc.scalar.dma_start(out=mv_d[isl, :], in_=mvt)
                    nc.vector.tensor_scalar(out=y1[i], in0=pre,
                                            scalar1=dinv[:, i:i + 1], scalar2=0.0,
                                            op0=OP.mult, op1=OP.max)

            # ---- A-products (bf16): tdst[db][:, c] = (adj @ y + y).T -------
            def a_product(ysrc, kb, tdst):
                for c in range(NCH):
                    c0 = c * CH
                    for db in range(kb):
                        pa = ps.tile([P, CH], f32, name="pa", tag="ptmp", bufs=4)
                        mms = []
                        for m in range(T):
                            mms.append((ysrc[m][:, db * P:(db + 1) * P],
                                        adjt[:, m, c0:c0 + CH], pa[:, :]))
                            if c0 <= m * P < c0 + CH:
                                off = m * P - c0
                                mms.append((ysrc[m][:, db * P:(db + 1) * P],
                                            eyeb, pa[:, off:off + P]))
                        for q, (l, r, o) in enumerate(mms):
                            nc.tensor.matmul(o, lhsT=l, rhs=r, start=(q == 0),
                                             stop=(q == len(mms) - 1))
                        if (c + db) % 2 == 0:
                            nc.vector.tensor_copy(tdst[db][:, c0:c0 + CH], pa)
                        else:
                            nc.scalar.copy(tdst[db][:, c0:c0 + CH], pa)

            t1T = [feat.tile([P, n], f32r, name=f"t1T{k}", tag="featbig")
                   for k in range(KD)]
            a_product(y1, KD, t1T)

            # ---- hid = relu(dinv*(t1@W1) + b1); y2 = bf16(dinv * hid) ------
            y2 = [persist.tile([P, dhid], bf16, name=f"y2_{i}", tag=f"y2_{i}")
                  for i in range(T)]
            for i in range(T):
                isl = slice(i * P, (i + 1) * P)
                ph = ps.tile([P, dhid], f32, name="ph", tag="pw", bufs=2)
                for k in range(KD):
                    nc.tensor.matmul(ph, lhsT=t1T[k][:, isl], rhs=w1_sb[k],
                                     start=(k == 0), stop=(k == KD - 1))
                pre1 = small.tile([P, dhid], f32, name="pre1", tag="pre")
                nc.vector.scalar_tensor_tensor(pre1, ph, dinv[:, i:i + 1], b1_sb,
                                               op0=OP.mult, op1=OP.add)
                hidt = small.tile([P, dhid], f32, name="hidt", tag="mvt")
                nc.scalar.activation(hidt, pre1, AF.Relu)
                nc.sync.dma_start(out=hid_d[isl, :], in_=hidt)
                nc.vector.tensor_scalar(out=y2[i], in0=pre1, scalar1=dinv[:, i:i + 1],
                                        scalar2=0.0, op0=OP.mult, op1=OP.max)

            t2T = [feat.tile([P, n], f32r, name=f"t2T{k}", tag="featbig")
                   for k in range(KH)]
            a_product(y2, KH, t2T)

            # ---- out = relu(dinv*(t2@W2) + b2) -----------------------------
            for i in range(T):
                isl = slice(i * P, (i + 1) * P)
                po = ps.tile([P, dout], f32, name="po", tag="pw", bufs=2)
                for k in range(KH):
                    nc.tensor.matmul(po, lhsT=t2T[k][:, isl], rhs=w2_sb[k],
                                     start=(k == 0), stop=(k == KH - 1))
                pre2 = small.tile([P, dout], f32, name="pre2", tag="pre")
                nc.vector.scalar_tensor_tensor(pre2, po, dinv[:, i:i + 1], b2_sb,
                                               op0=OP.mult, op1=OP.add)
                outt = small.tile([P, dout], f32, name="outt", tag="mvt")
                nc.scalar.activation(outt, pre2, AF.Relu)
                nc.sync.dma_start(out=out_d[isl, :], in_=outt)

    nc.compile()
    return nc


VARIANT = "bf16"
_NC_CACHE = None


def _get_nc():
    global _NC_CACHE
    if _NC_CACHE is None:
        _NC_CACHE = _build_module_bf16()
    return _NC_CACHE


def _make_in_maps(adj, x, w_mean, b_mean, w1, b1, w2, b2):
    import ml_dtypes

    adj = np.asarray(adj, dtype=np.float32)
    x = np.asarray(x, dtype=np.float32)
    wm = np.ascontiguousarray(np.asarray(w_mean, dtype=np.float32))
    w1 = np.ascontiguousarray(np.asarray(w1, dtype=np.float32))
    w2 = np.ascontiguousarray(np.asarray(w2, dtype=np.float32))
    bm = np.ascontiguousarray(np.broadcast_to(np.asarray(b_mean, np.float32), (P, IN_DIM)))
    b1b = np.ascontiguousarray(np.broadcast_to(np.asarray(b1, np.float32), (P, HID_DIM)))
    b2b = np.ascontiguousarray(np.broadcast_to(np.asarray(b2, np.float32), (P, OUT_DIM)))
    eyeb = np.eye(P, dtype=ml_dtypes.bfloat16)
    eyef = np.eye(P, dtype=np.float32)
    onesb = np.ones((P, 1), dtype=ml_dtypes.bfloat16)
    onef = np.ones((1, 1), dtype=np.float32)
    return [
        dict(adjb=np.ascontiguousarray(adj[b].astype(ml_dtypes.bfloat16)),
             x=np.ascontiguousarray(x[b]),
             wm=wm, w1=w1, w2=w2, bm=bm, b1=b1b, b2=b2b,
             eyeb=eyeb, eyef=eyef, onesb=onesb, onef=onef)
        for b in range(B)
    ]


def kernel(adj, gcn_inputs, w_mean, b_mean, w1, b1, w2, b2):
    from concourse.bass_utils import run_bass_kernel_spmd

    nc = _get_nc()
    in_maps = _make_in_maps(adj, gcn_inputs, w_mean, b_mean, w1, b1, w2, b2)
    res = run_bass_kernel_spmd(nc, in_maps, core_ids=list(range(B)))
    mv = np.stack([res.results[b]["mv"] for b in range(B)])
    hid = np.stack([res.results[b]["hid"] for b in range(B)])
    out = np.stack([res.results[b]["out"] for b in range(B)])
    x = np.asarray(gcn_inputs, dtype=np.float32)
    return ((x, mv, hid, out), ())


# revision 20
# speedup vs baseline: 2.3255x; 1.0815x over previous
"""GCN (CGCN) forward kernel for Trainium2, data-parallel over batch on 8 NeuronCores.

Per core (one batch sample):
  d      = adj.sum(-1) + 1 ;  dinv = d**-0.5
  mv     = relu(x @ Wm + bm)
  t1     = adj @ (dinv*mv) + (dinv*mv)          # A @ Dinv @ mv  with A = adj + I
  hid    = relu(dinv*(t1 @ W1) + b1)
  t2     = adj @ (dinv*hid) + (dinv*hid)
  out    = relu(dinv*(t2 @ W2) + b2)

adj is shipped to the device in bf16 and transposed by the DMA xbar engine
straight from DRAM into a resident SBUF tensor (the PE contracts over the
partition axis, so adj@v needs adj^T tiles).  Row sums come from PE
ones-products over the transposed tiles, so adj is read from HBM exactly once.
All matmuls run in bf16 with fp32 PSUM accumulation; everything else
(normalization, biases, relu, outputs) stays fp32.
"""

import numpy as np

B, N, IN_DIM, HID_DIM, OUT_DIM = 8, 2048, 256, 256, 128
P = 128


def _build_module_bf16(n=N, din=IN_DIM, dhid=HID_DIM, dout=OUT_DIM):
    from contextlib import ExitStack

    import concourse.mybir as mybir
    import concourse.tile as tile
    from concourse import bacc

    f32 = mybir.dt.float32
    f32r = mybir.dt.float32r
    bf16 = mybir.dt.bfloat16
    OP = mybir.AluOpType
    AF = mybir.ActivationFunctionType

    T = n // P
    KD = din // P
    KH = dhid // P
    CH = min(512, n)
    NCH = n // CH
    TC_ = CH // P  # stream tiles per chunk

    nc = bacc.Bacc(None, target_bir_lowering=False)

    adjb_d = nc.declare_dram_parameter("adjb", [n, n], bf16, isOutput=False)
    x_d = nc.declare_dram_parameter("x", [n, din], f32, isOutput=False)
    wm_d = nc.declare_dram_parameter("wm", [din, din], f32, isOutput=False)
    w1_d = nc.declare_dram_parameter("w1", [din, dhid], f32, isOutput=False)
    w2_d = nc.declare_dram_parameter("w2", [dhid, dout], f32, isOutput=False)
    bm_d = nc.declare_dram_parameter("bm", [P, din], f32, isOutput=False)
    b1_d = nc.declare_dram_parameter("b1", [P, dhid], f32, isOutput=False)
    b2_d = nc.declare_dram_parameter("b2", [P, dout], f32, isOutput=False)
    eyeb_d = nc.declare_dram_parameter("eyeb", [P, P], bf16, isOutput=False)
    eyef_d = nc.declare_dram_parameter("eyef", [P, P], f32, isOutput=False)
    onesb_d = nc.declare_dram_parameter("onesb", [P, 1], bf16, isOutput=False)
    onef_d = nc.declare_dram_parameter("onef", [1, 1], f32, isOutput=False)
    mv_d = nc.declare_dram_parameter("mv", [n, din], f32, isOutput=True)
    hid_d = nc.declare_dram_parameter("hid", [n, dhid], f32, isOutput=True)
    out_d = nc.declare_dram_parameter("out", [n, dout], f32, isOutput=True)

    with tile.TileContext(nc) as tc:
        with ExitStack() as ctx:
            persist = ctx.enter_context(tc.tile_pool(name="persist", bufs=1))
            small = ctx.enter_context(tc.tile_pool(name="small", bufs=4))
            feat = ctx.enter_context(tc.tile_pool(name="feat", bufs=2))
            ps = ctx.enter_context(tc.tile_pool(name="ps", bufs=2, space="PSUM"))

            # ---- constants (all regular DMAs issue BEFORE the xbar transposes
            # to avoid the DMATranspose<->DMACopy serialization ping-pong) ----
            eyeb = persist.tile([P, P], bf16, name="eyeb", tag="eyeb")
            nc.gpsimd.dma_start(out=eyeb, in_=eyeb_d[:, :])
            eyer = persist.tile([P, P], f32r, name="eyer", tag="eyer")
            nc.gpsimd.dma_start(out=eyer, in_=eyef_d[:, :])
            onesb = persist.tile([P, 1], bf16, name="onesb", tag="onesb")
            nc.gpsimd.dma_start(out=onesb, in_=onesb_d[:, :])
            onef = persist.tile([1, 1], f32, name="onef", tag="onef")
            nc.gpsimd.dma_start(out=onef, in_=onef_d[:, :])
            wm_all = persist.tile([P, KD, din], f32r, name="wm_all", tag="wm_all")
            nc.gpsimd.dma_start(out=wm_all, in_=wm_d.rearrange("(k p) f -> p k f", p=P))
            w1_all = persist.tile([P, KD, dhid], f32r, name="w1_all", tag="w1_all")
            nc.gpsimd.dma_start(out=w1_all, in_=w1_d.rearrange("(k p) f -> p k f", p=P))
            w2_all = persist.tile([P, KH, dout], f32r, name="w2_all", tag="w2_all")
            nc.gpsimd.dma_start(out=w2_all, in_=w2_d.rearrange("(k p) f -> p k f", p=P))
            wm_sb = [wm_all[:, k, :] for k in range(KD)]
            w1_sb = [w1_all[:, k, :] for k in range(KD)]
            w2_sb = [w2_all[:, k, :] for k in range(KH)]
            bm_sb = persist.tile([P, din], f32, name="bm_sb", tag="bm_sb")
            nc.gpsimd.dma_start(out=bm_sb, in_=bm_d[:, :])
            b1_sb = persist.tile([P, dhid], f32, name="b1_sb", tag="b1_sb")
            nc.gpsimd.dma_start(out=b1_sb, in_=b1_d[:, :])
            b2_sb = persist.tile([P, dout], f32, name="b2_sb", tag="b2_sb")
            nc.gpsimd.dma_start(out=b2_sb, in_=b2_d[:, :])
            # whole x in one cast-DMA: xb[p, i, f] = x[i*128+p, f] as f32r
            xb_all = persist.tile([P, T, din], f32r, name="xb_all", tag="xb_all")
            nc.gpsimd.dma_start(out=xb_all, in_=x_d.rearrange("(i p) f -> p i f", p=P))

            dinv = persist.tile([P, T], f32, name="dinv", tag="dinv")

            # adjT resident in SBUF: adjt[mp, j, nf] = adj[nf_global, j*128+mp]
            adjt = persist.tile([P, T, n], bf16, name="adjt", tag="adjt")

            xT = [persist.tile([P, n], f32r, name=f"xT{k}", tag=f"xT{k}")
                  for k in range(KD)]
            y1 = [persist.tile([P, din], bf16, name=f"y1_{i}", tag=f"y1_{i}")
                  for i in range(T)]

            # ---- all 16 xbar transposes back-to-back (nothing else between) -
            for i in range(T):
                isl = slice(i * P, (i + 1) * P)
                nc.sync.dma_start_transpose(adjt[:, :, isl], adjb_d[isl, :])

            # ---- per chunk: d/dinv, xT, Wm, y1 (PE chases the transposes) --
            for c in range(NCH):
                tiles = range(c * TC_, (c + 1) * TC_)
                # d for output rows of this chunk: ones^T-product over adjT
                csl = slice(c * CH, (c + 1) * CH)
                pdr = ps.tile([1, CH], f32, name="pdr", tag="pdrow", bufs=2)
                for j in range(T):
                    nc.tensor.matmul(pdr, lhsT=onesb, rhs=adjt[:, j, csl],
                                     start=(j == 0), stop=(j == T - 1))
                drow = small.tile([1, CH], f32, name="drow", tag="drow")
                nc.vector.tensor_copy(drow, pdr)
                for q, i in enumerate(tiles):
                    pcol = ps.tile([P, 1], f32, name="pcol", tag="pdrow", bufs=2)
                    nc.tensor.matmul(pcol, lhsT=drow[0:1, q * P:(q + 1) * P],
                                     rhs=onef, start=True, stop=True)
                    srt = small.tile([P, 1], f32, name="srt", tag="srt")
                    nc.scalar.activation(srt, pcol, AF.Sqrt, bias=1.0)
                    nc.vector.reciprocal(dinv[:, i:i + 1], srt)

                for i in tiles:
                    isl = slice(i * P, (i + 1) * P)
                    # xT via PE transpose (f32r identity matmul)
                    for k in range(KD):
                        ptx = ps.tile([P, P], f32, name="ptx", tag="ptmp", bufs=4)
                        nc.tensor.matmul(ptx, lhsT=xb_all[:, i, k * P:(k + 1) * P],
                                         rhs=eyer, start=True, stop=True)
                        if (i + k) % 2 == 0:
                            nc.vector.tensor_copy(xT[k][:, isl], ptx)
                        else:
                            nc.scalar.copy(xT[k][:, isl], ptx)
                    # mv = relu(x@Wm + bm); y1 = bf16(dinv * mv)
                    pm = ps.tile([P, din], f32, name="pm", tag="pw", bufs=2)
                    for k in range(KD):
                        nc.tensor.matmul(pm, lhsT=xT[k][:, isl], rhs=wm_sb[k],
                                         start=(k == 0), stop=(k == KD - 1))
                    pre = small.tile([P, din], f32, name="pre", tag="pre")
                    nc.vector.tensor_add(pre, pm, bm_sb)
                    mvt = small.tile([P, din], f32, name="mvt", tag="mvt", bufs=16)
                    nc.scalar.activation(mvt, pre, AF.Relu)
                    nc.scalar.dma_start(out=mv_d[isl, :], in_=mvt)
                    nc.vector.tensor_scalar(out=y1[i], in0=pre,
                                            scalar1=dinv[:, i:i + 1], scalar2=0.0,
                                            op0=OP.mult, op1=OP.max)

            # ---- A-products (bf16): tdst[db][:, c] = (adj @ y + y).T -------
            def a_product(ysrc, kb, tdst):
                for c in range(NCH):
                    c0 = c * CH
                    for db in range(kb):
                        pa = ps.tile([P, CH], f32, name="pa", tag="ptmp", bufs=4)
                        mms = []
                        for m in range(T):
                            mms.append((ysrc[m][:, db * P:(db + 1) * P],
                                        adjt[:, m, c0:c0 + CH], pa[:, :]))
                            if c0 <= m * P < c0 + CH:
                                off = m * P - c0
                                mms.append((ysrc[m][:, db * P:(db + 1) * P],
                                            eyeb, pa[:, off:off + P]))
                        for q, (l, r, o) in enumerate(mms):
                            nc.tensor.matmul(o, lhsT=l, rhs=r, start=(q == 0),
                                             stop=(q == len(mms) - 1))
                        if (c + db) % 2 == 0:
                            nc.vector.tensor_copy(tdst[db][:, c0:c0 + CH], pa)
                        else:
                            nc.scalar.copy(tdst[db][:, c0:c0 + CH], pa)

            t1T = [feat.tile([P, n], f32r, name=f"t1T{k}", tag="featbig")
                   for k in range(KD)]
            a_product(y1, KD, t1T)

            # ---- hid = relu(dinv*(t1@W1) + b1); y2 = bf16(dinv * hid) ------
            y2 = [persist.tile([P, dhid], bf16, name=f"y2_{i}", tag=f"y2_{i}")
                  for i in range(T)]
            for i in range(T):
                isl = slice(i * P, (i + 1) * P)
                ph = ps.tile([P, dhid], f32, name="ph", tag="pw", bufs=2)
                for k in range(KD):
                    nc.tensor.matmul(ph, lhsT=t1T[k][:, isl], rhs=w1_sb[k],
                                     start=(k == 0), stop=(k == KD - 1))
                pre1 = small.tile([P, dhid], f32, name="pre1", tag="pre")
                nc.vector.scalar_tensor_tensor(pre1, ph, dinv[:, i:i + 1], b1_sb,
                                               op0=OP.mult, op1=OP.add)
                hidt = small.tile([P, dhid], f32, name="hidt", tag="hot")
                nc.scalar.activation(hidt, pre1, AF.Relu)
                nc.sync.dma_start(out=hid_d[isl, :], in_=hidt)
                nc.vector.tensor_scalar(out=y2[i], in0=pre1, scalar1=dinv[:, i:i + 1],
                                        scalar2=0.0, op0=OP.mult, op1=OP.max)

            t2T = [feat.tile([P, n], f32r, name=f"t2T{k}", tag="featbig")
                   for k in range(KH)]
            a_product(y2, KH, t2T)

            # ---- out = relu(dinv*(t2@W2) + b2) -----------------------------
            for i in range(T):
                isl = slice(i * P, (i + 1) * P)
                po = ps.tile([P, dout], f32, name="po", tag="pw", bufs=2)
                for k in range(KH):
                    nc.tensor.matmul(po, lhsT=t2T[k][:, isl], rhs=w2_sb[k],
                                     start=(k == 0), stop=(k == KH - 1))
                pre2 = small.tile([P, dout], f32, name="pre2", tag="pre")
                nc.vector.scalar_tensor_tensor(pre2, po, dinv[:, i:i + 1], b2_sb,
                                               op0=OP.mult, op1=OP.add)
                outt = small.tile([P, dout], f32, name="outt", tag="hot")
                nc.scalar.activation(outt, pre2, AF.Relu)
                nc.sync.dma_start(out=out_d[isl, :], in_=outt)

    nc.compile()
    return nc


VARIANT = "bf16"
_NC_CACHE = None


def _get_nc():
    global _NC_CACHE
    if _NC_CACHE is None:
        _NC_CACHE = _build_module_bf16()
    return _NC_CACHE


def _make_in_maps(adj, x, w_mean, b_mean, w1, b1, w2, b2):
    import ml_dtypes

    adj = np.asarray(adj, dtype=np.float32)
    x = np.asarray(x, dtype=np.float32)
    wm = np.ascontiguousarray(np.asarray(w_mean, dtype=np.float32))
    w1 = np.ascontiguousarray(np.asarray(w1, dtype=np.float32))
    w2 = np.ascontiguousarray(np.asarray(w2, dtype=np.float32))
    bm = np.ascontiguousarray(np.broadcast_to(np.asarray(b_mean, np.float32), (P, IN_DIM)))
    b1b = np.ascontiguousarray(np.broadcast_to(np.asarray(b1, np.float32), (P, HID_DIM)))
    b2b = np.ascontiguousarray(np.broadcast_to(np.asarray(b2, np.float32), (P, OUT_DIM)))
    eyeb = np.eye(P, dtype=ml_dtypes.bfloat16)
    eyef = np.eye(P, dtype=np.float32)
    onesb = np.ones((P, 1), dtype=ml_dtypes.bfloat16)
    onef = np.ones((1, 1), dtype=np.float32)
    return [
        dict(adjb=np.ascontiguousarray(adj[b].astype(ml_dtypes.bfloat16)),
             x=np.ascontiguousarray(x[b]),
             wm=wm, w1=w1, w2=w2, bm=bm, b1=b1b, b2=b2b,
             eyeb=eyeb, eyef=eyef, onesb=onesb, onef=onef)
        for b in range(B)
    ]


def kernel(adj, gcn_inputs, w_mean, b_mean, w1, b1, w2, b2):
    from concourse.bass_utils import run_bass_kernel_spmd

    nc = _get_nc()
    in_maps = _make_in_maps(adj, gcn_inputs, w_mean, b_mean, w1, b1, w2, b2)
    res = run_bass_kernel_spmd(nc, in_maps, core_ids=list(range(B)))
    mv = np.stack([res.results[b]["mv"] for b in range(B)])
    hid = np.stack([res.results[b]["hid"] for b in range(B)])
    out = np.stack([res.results[b]["out"] for b in range(B)])
    x = np.asarray(gcn_inputs, dtype=np.float32)
    return ((x, mv, hid, out), ())


# revision 21
# speedup vs baseline: 2.3297x; 1.0018x over previous
"""GCN (CGCN) forward kernel for Trainium2, data-parallel over batch on 8 NeuronCores.

Per core (one batch sample):
  d      = adj.sum(-1) + 1 ;  dinv = d**-0.5
  mv     = relu(x @ Wm + bm)
  t1     = adj @ (dinv*mv) + (dinv*mv)          # A @ Dinv @ mv  with A = adj + I
  hid    = relu(dinv*(t1 @ W1) + b1)
  t2     = adj @ (dinv*hid) + (dinv*hid)
  out    = relu(dinv*(t2 @ W2) + b2)

adj is shipped to the device in bf16 and transposed by the DMA xbar engine
straight from DRAM into a resident SBUF tensor (the PE contracts over the
partition axis, so adj@v needs adj^T tiles).  Row sums come from PE
ones-products over the transposed tiles, so adj is read from HBM exactly once.
All matmuls run in bf16 with fp32 PSUM accumulation; everything else
(normalization, biases, relu, outputs) stays fp32.
"""

import numpy as np

B, N, IN_DIM, HID_DIM, OUT_DIM = 8, 2048, 256, 256, 128
P = 128


def _build_module_bf16(n=N, din=IN_DIM, dhid=HID_DIM, dout=OUT_DIM):
    from contextlib import ExitStack

    import concourse.mybir as mybir
    import concourse.tile as tile
    from concourse import bacc

    f32 = mybir.dt.float32
    f32r = mybir.dt.float32r
    bf16 = mybir.dt.bfloat16
    OP = mybir.AluOpType
    AF = mybir.ActivationFunctionType

    T = n // P
    KD = din // P
    KH = dhid // P
    CH = min(512, n)
    NCH = n // CH
    TC_ = CH // P  # stream tiles per chunk

    nc = bacc.Bacc(None, target_bir_lowering=False)

    adjb_d = nc.declare_dram_parameter("adjb", [n, n], bf16, isOutput=False)
    x_d = nc.declare_dram_parameter("x", [n, din], f32r, isOutput=False)
    wm_d = nc.declare_dram_parameter("wm", [din, din], f32r, isOutput=False)
    w1_d = nc.declare_dram_parameter("w1", [din, dhid], f32r, isOutput=False)
    w2_d = nc.declare_dram_parameter("w2", [dhid, dout], f32r, isOutput=False)
    bm_d = nc.declare_dram_parameter("bm", [P, din], f32, isOutput=False)
    b1_d = nc.declare_dram_parameter("b1", [P, dhid], f32, isOutput=False)
    b2_d = nc.declare_dram_parameter("b2", [P, dout], f32, isOutput=False)
    eyeb_d = nc.declare_dram_parameter("eyeb", [P, P], bf16, isOutput=False)
    eyef_d = nc.declare_dram_parameter("eyef", [P, P], f32r, isOutput=False)
    onesb_d = nc.declare_dram_parameter("onesb", [P, 1], bf16, isOutput=False)
    onef_d = nc.declare_dram_parameter("onef", [1, 1], f32, isOutput=False)
    mv_d = nc.declare_dram_parameter("mv", [n, din], f32, isOutput=True)
    hid_d = nc.declare_dram_parameter("hid", [n, dhid], f32, isOutput=True)
    out_d = nc.declare_dram_parameter("out", [n, dout], f32, isOutput=True)

    with tile.TileContext(nc) as tc:
        with ExitStack() as ctx:
            persist = ctx.enter_context(tc.tile_pool(name="persist", bufs=1))
            small = ctx.enter_context(tc.tile_pool(name="small", bufs=4))
            feat = ctx.enter_context(tc.tile_pool(name="feat", bufs=2))
            ps = ctx.enter_context(tc.tile_pool(name="ps", bufs=2, space="PSUM"))

            # ---- constants (all regular DMAs issue BEFORE the xbar transposes
            # to avoid the DMATranspose<->DMACopy serialization ping-pong) ----
            eyeb = persist.tile([P, P], bf16, name="eyeb", tag="eyeb")
            nc.scalar.dma_start(out=eyeb, in_=eyeb_d[:, :])
            eyer = persist.tile([P, P], f32r, name="eyer", tag="eyer")
            nc.scalar.dma_start(out=eyer, in_=eyef_d[:, :])
            onesb = persist.tile([P, 1], bf16, name="onesb", tag="onesb")
            nc.scalar.dma_start(out=onesb, in_=onesb_d[:, :])
            onef = persist.tile([1, 1], f32, name="onef", tag="onef")
            nc.scalar.dma_start(out=onef, in_=onef_d[:, :])
            wm_sb, w1_sb, w2_sb = [], [], []
            for k in range(KD):
                t = persist.tile([P, din], f32r, name=f"wm{k}", tag=f"wm{k}")
                nc.scalar.dma_start(out=t, in_=wm_d[k * P:(k + 1) * P, :])
                wm_sb.append(t)
            for k in range(KD):
                t = persist.tile([P, dhid], f32r, name=f"w1_{k}", tag=f"w1_{k}")
                nc.scalar.dma_start(out=t, in_=w1_d[k * P:(k + 1) * P, :])
                w1_sb.append(t)
            for k in range(KH):
                t = persist.tile([P, dout], f32r, name=f"w2_{k}", tag=f"w2_{k}")
                nc.scalar.dma_start(out=t, in_=w2_d[k * P:(k + 1) * P, :])
                w2_sb.append(t)
            bm_sb = persist.tile([P, din], f32, name="bm_sb", tag="bm_sb")
            nc.scalar.dma_start(out=bm_sb, in_=bm_d[:, :])
            b1_sb = persist.tile([P, dhid], f32, name="b1_sb", tag="b1_sb")
            nc.scalar.dma_start(out=b1_sb, in_=b1_d[:, :])
            b2_sb = persist.tile([P, dout], f32, name="b2_sb", tag="b2_sb")
            nc.scalar.dma_start(out=b2_sb, in_=b2_d[:, :])
            # whole x resident, 16 contiguous tile loads
            xb_all = persist.tile([P, T, din], f32r, name="xb_all", tag="xb_all")
            for i in range(T):
                nc.scalar.dma_start(out=xb_all[:, i, :],
                                    in_=x_d[i * P:(i + 1) * P, :])

            dinv = persist.tile([P, T], f32, name="dinv", tag="dinv")

            # adjT resident in SBUF: adjt[mp, j, nf] = adj[nf_global, j*128+mp]
            adjt = persist.tile([P, T, n], bf16, name="adjt", tag="adjt")

            xT = [persist.tile([P, n], f32r, name=f"xT{k}", tag=f"xT{k}")
                  for k in range(KD)]
            y1 = [persist.tile([P, din], bf16, name=f"y1_{i}", tag=f"y1_{i}")
                  for i in range(T)]

            # ---- all 16 xbar transposes back-to-back (nothing else between) -
            for i in range(T):
                isl = slice(i * P, (i + 1) * P)
                nc.sync.dma_start_transpose(adjt[:, :, isl], adjb_d[isl, :])

            # ---- per chunk: d/dinv, xT, Wm, y1 (PE chases the transposes) --
            for c in range(NCH):
                tiles = range(c * TC_, (c + 1) * TC_)
                # d for output rows of this chunk: ones^T-product over adjT
                csl = slice(c * CH, (c + 1) * CH)
                pdr = ps.tile([1, CH], f32, name="pdr", tag="pdrow", bufs=2)
                for j in range(T):
                    nc.tensor.matmul(pdr, lhsT=onesb, rhs=adjt[:, j, csl],
                                     start=(j == 0), stop=(j == T - 1))
                drow = small.tile([1, CH], f32, name="drow", tag="drow")
                nc.vector.tensor_copy(drow, pdr)
                for q, i in enumerate(tiles):
                    pcol = ps.tile([P, 1], f32, name="pcol", tag="pdrow", bufs=2)
                    nc.tensor.matmul(pcol, lhsT=drow[0:1, q * P:(q + 1) * P],
                                     rhs=onef, start=True, stop=True)
                    srt = small.tile([P, 1], f32, name="srt", tag="srt")
                    nc.scalar.activation(srt, pcol, AF.Sqrt, bias=1.0)
                    nc.vector.reciprocal(dinv[:, i:i + 1], srt)

                for i in tiles:
                    isl = slice(i * P, (i + 1) * P)
                    # xT via PE transpose (f32r identity matmul)
                    for k in range(KD):
                        ptx = ps.tile([P, P], f32, name="ptx", tag="ptmp", bufs=4)
                        nc.tensor.matmul(ptx, lhsT=xb_all[:, i, k * P:(k + 1) * P],
                                         rhs=eyer, start=True, stop=True)
                        if (i + k) % 2 == 0:
                            nc.vector.tensor_copy(xT[k][:, isl], ptx)
                        else:
                            nc.scalar.copy(xT[k][:, isl], ptx)
                    # mv = relu(x@Wm + bm); y1 = bf16(dinv * mv)
                    pm = ps.tile([P, din], f32, name="pm", tag="pw", bufs=2)
                    for k in range(KD):
                        nc.tensor.matmul(pm, lhsT=xT[k][:, isl], rhs=wm_sb[k],
                                         start=(k == 0), stop=(k == KD - 1))
                    pre = small.tile([P, din], f32, name="pre", tag="pre")
                    nc.vector.tensor_add(pre, pm, bm_sb)
                    mvt = small.tile([P, din], f32, name="mvt", tag="mvt", bufs=16)
                    nc.scalar.activation(mvt, pre, AF.Relu)
                    nc.sync.dma_start(out=mv_d[isl, :], in_=mvt)
                    nc.vector.tensor_scalar(out=y1[i], in0=pre,
                                            scalar1=dinv[:, i:i + 1], scalar2=0.0,
                                            op0=OP.mult, op1=OP.max)

            # ---- A-products (bf16): tdst[db][:, c] = (adj @ y + y).T -------
            def a_product(ysrc, kb, tdst):
                for c in range(NCH):
                    c0 = c * CH
                    for db in range(kb):
                        pa = ps.tile([P, CH], f32, name="pa", tag="ptmp", bufs=4)
                        mms = []
                        for m in range(T):
                            mms.append((ysrc[m][:, db * P:(db + 1) * P],
                                        adjt[:, m, c0:c0 + CH], pa[:, :]))
                            if c0 <= m * P < c0 + CH:
                                off = m * P - c0
                                mms.append((ysrc[m][:, db * P:(db + 1) * P],
                                            eyeb, pa[:, off:off + P]))
                        for q, (l, r, o) in enumerate(mms):
                            nc.tensor.matmul(o, lhsT=l, rhs=r, start=(q == 0),
                                             stop=(q == len(mms) - 1))
                        if (c + db) % 2 == 0:
                            nc.vector.tensor_copy(tdst[db][:, c0:c0 + CH], pa)
                        else:
                            nc.scalar.copy(tdst[db][:, c0:c0 + CH], pa)

            t1T = [feat.tile([P, n], f32r, name=f"t1T{k}", tag="featbig")
                   for k in range(KD)]
            a_product(y1, KD, t1T)

            # ---- hid = relu(dinv*(t1@W1) + b1); y2 = bf16(dinv * hid) ------
            y2 = [persist.tile([P, dhid], bf16, name=f"y2_{i}", tag=f"y2_{i}")
                  for i in range(T)]
            for i in range(T):
                isl = slice(i * P, (i + 1) * P)
                ph = ps.tile([P, dhid], f32, name="ph", tag="pw", bufs=2)
                for k in range(KD):
                    nc.tensor.matmul(ph, lhsT=t1T[k][:, isl], rhs=w1_sb[k],
                                     start=(k == 0), stop=(k == KD - 1))
                pre1 = small.tile([P, dhid], f32, name="pre1", tag="pre")
                nc.vector.scalar_tensor_tensor(pre1, ph, dinv[:, i:i + 1], b1_sb,
                                               op0=OP.mult, op1=OP.add)
                hidt = small.tile([P, dhid], f32, name="hidt", tag="hot")
                nc.scalar.activation(hidt, pre1, AF.Relu)
                nc.sync.dma_start(out=hid_d[isl, :], in_=hidt)
                nc.vector.tensor_scalar(out=y2[i], in0=pre1, scalar1=dinv[:, i:i + 1],
                                        scalar2=0.0, op0=OP.mult, op1=OP.max)

            t2T = [feat.tile([P, n], f32r, name=f"t2T{k}", tag="featbig")
                   for k in range(KH)]
            a_product(y2, KH, t2T)

            # ---- out = relu(dinv*(t2@W2) + b2) -----------------------------
            for i in range(T):
                isl = slice(i * P, (i + 1) * P)
                po = ps.tile([P, dout], f32, name="po", tag="pw", bufs=2)
                for k in range(KH):
                    nc.tensor.matmul(po, lhsT=t2T[k][:, isl], rhs=w2_sb[k],
                                     start=(k == 0), stop=(k == KH - 1))
                pre2 = small.tile([P, dout], f32, name="pre2", tag="pre")
                nc.vector.scalar_tensor_tensor(pre2, po, dinv[:, i:i + 1], b2_sb,
                                               op0=OP.mult, op1=OP.add)
                outt = small.tile([P, dout], f32, name="outt", tag="hot")
                nc.scalar.activation(outt, pre2, AF.Relu)
                nc.sync.dma_start(out=out_d[isl, :], in_=outt)

    nc.compile()
    return nc


VARIANT = "bf16"
_NC_CACHE = None


def _get_nc():
    global _NC_CACHE
    if _NC_CACHE is None:
        _NC_CACHE = _build_module_bf16()
    return _NC_CACHE


def _make_in_maps(adj, x, w_mean, b_mean, w1, b1, w2, b2):
    import ml_dtypes

    adj = np.asarray(adj, dtype=np.float32)
    x = np.asarray(x, dtype=np.float32)
    wm = np.ascontiguousarray(np.asarray(w_mean, dtype=np.float32))
    w1 = np.ascontiguousarray(np.asarray(w1, dtype=np.float32))
    w2 = np.ascontiguousarray(np.asarray(w2, dtype=np.float32))
    bm = np.ascontiguousarray(np.broadcast_to(np.asarray(b_mean, np.float32), (P, IN_DIM)))
    b1b = np.ascontiguousarray(np.broadcast_to(np.asarray(b1, np.float32), (P, HID_DIM)))
    b2b = np.ascontiguousarray(np.broadcast_to(np.asarray(b2, np.float32), (P, OUT_DIM)))
    eyeb = np.eye(P, dtype=ml_dtypes.bfloat16)
    eyef = np.eye(P, dtype=np.float32)
    onesb = np.ones((P, 1), dtype=ml_dtypes.bfloat16)
    onef = np.ones((1, 1), dtype=np.float32)
    return [
        dict(adjb=np.ascontiguousarray(adj[b].astype(ml_dtypes.bfloat16)),
             x=np.ascontiguousarray(x[b]),
             wm=wm, w1=w1, w2=w2, bm=bm, b1=b1b, b2=b2b,
             eyeb=eyeb, eyef=eyef, onesb=onesb, onef=onef)
        for b in range(B)
    ]


def kernel(adj, gcn_inputs, w_mean, b_mean, w1, b1, w2, b2):
    from concourse.bass_utils import run_bass_kernel_spmd

    nc = _get_nc()
    in_maps = _make_in_maps(adj, gcn_inputs, w_mean, b_mean, w1, b1, w2, b2)
    res = run_bass_kernel_spmd(nc, in_maps, core_ids=list(range(B)))
    mv = np.stack([res.results[b]["mv"] for b in range(B)])
    hid = np.stack([res.results[b]["hid"] for b in range(B)])
    out = np.stack([res.results[b]["out"] for b in range(B)])
    x = np.asarray(gcn_inputs, dtype=np.float32)
    return ((x, mv, hid, out), ())


# revision 35
# speedup vs baseline: 2.4465x; 1.0501x over previous
"""GCN (CGCN) forward kernel for Trainium2, data-parallel over batch on 8 NeuronCores.

Per core (one batch sample):
  d      = adj.sum(-1) + 1 ;  dinv = d**-0.5
  mv     = relu(x @ Wm + bm)
  t1     = adj @ (dinv*mv) + (dinv*mv)          # A @ Dinv @ mv  with A = adj + I
  hid    = relu(dinv*(t1 @ W1) + b1)
  t2     = adj @ (dinv*hid) + (dinv*hid)
  out    = relu(dinv*(t2 @ W2) + b2)

adj is shipped to the device in bf16 and transposed by the DMA xbar engine
straight from DRAM into a resident SBUF tensor (the PE contracts over the
partition axis, so adj@v needs adj^T tiles).  Row sums come from PE
ones-products over the transposed tiles, so adj is read from HBM exactly once.
All matmuls run in bf16 with fp32 PSUM accumulation; everything else
(normalization, biases, relu, outputs) stays fp32.
"""

import numpy as np

B, N, IN_DIM, HID_DIM, OUT_DIM = 8, 2048, 256, 256, 128
P = 128


def _build_module_bf16(n=N, din=IN_DIM, dhid=HID_DIM, dout=OUT_DIM, dbg=False):
    from contextlib import ExitStack

    import concourse.mybir as mybir
    import concourse.tile as tile
    from concourse import bacc

    f32 = mybir.dt.float32
    f32r = mybir.dt.float32r
    bf16 = mybir.dt.bfloat16
    OP = mybir.AluOpType
    AF = mybir.ActivationFunctionType

    T = n // P
    KD = din // P
    KH = dhid // P
    CH = min(512, n)
    NCH = n // CH
    TC_ = CH // P  # stream tiles per chunk

    nc = bacc.Bacc(None, target_bir_lowering=False)

    dcat = din + dhid + dout
    adjb_d = nc.declare_dram_parameter("adjb", [n, n], bf16, isOutput=False)
    x_d = nc.declare_dram_parameter("x", [n, din], f32r, isOutput=False)
    w01_d = nc.declare_dram_parameter("w01", [din, dcat], f32r, isOutput=False)
    bias_d = nc.declare_dram_parameter("bias", [P, dcat], f32, isOutput=False)
    eyeb_d = nc.declare_dram_parameter("eyeb", [P, P], bf16, isOutput=False)
    onesb_d = nc.declare_dram_parameter("onesb", [P, 2], bf16, isOutput=False)
    eyef_d = nc.declare_dram_parameter("eyef", [P, P], f32r, isOutput=False)
    onef_d = nc.declare_dram_parameter("onef", [1, 1], f32, isOutput=False)
    mv_d = nc.declare_dram_parameter("mv", [n, din], f32, isOutput=True)
    hid_d = nc.declare_dram_parameter("hid", [n, dhid], f32, isOutput=True)
    out_d = nc.declare_dram_parameter("out", [n, dout], f32, isOutput=True)
    if dbg:
        y1_dbg = nc.declare_dram_parameter("y1_dbg", [T, P, din], f32, isOutput=True)
        t1_dbg = nc.declare_dram_parameter("t1_dbg", [KD, P, n], f32, isOutput=True)
        dinv_dbg = nc.declare_dram_parameter("dinv_dbg", [P, T], f32, isOutput=True)
        adjt_dbg = nc.declare_dram_parameter("adjt_dbg", [4, P, n], bf16, isOutput=True)

    with tile.TileContext(nc) as tc:
        with ExitStack() as ctx:
            persist = ctx.enter_context(tc.tile_pool(name="persist", bufs=1))
            small = ctx.enter_context(tc.tile_pool(name="small", bufs=4))
            feat = ctx.enter_context(tc.tile_pool(name="feat", bufs=2))
            ps = ctx.enter_context(tc.tile_pool(name="ps", bufs=2, space="PSUM"))

            # ---- all loads on the SP ring, BEFORE the xbar transposes ----
            load_insts = []
            eyeb = persist.tile([P, P], bf16, name="eyeb", tag="eyeb")
            load_insts.append(nc.sync.dma_start(out=eyeb, in_=eyeb_d[:, :]))
            onesb2 = persist.tile([P, 2], bf16, name="onesb2", tag="onesb2")
            load_insts.append(nc.sync.dma_start(out=onesb2, in_=onesb_d[:, :]))
            onesb = onesb2[:, 0:1]
            eyer = persist.tile([P, P], f32r, name="eyer", tag="eyer")
            load_insts.append(nc.sync.dma_start(out=eyer, in_=eyef_d[:, :]))
            onef = persist.tile([1, 1], f32, name="onef", tag="onef")
            load_insts.append(nc.sync.dma_start(out=onef, in_=onef_d[:, :]))
            wm_sb, w1_sb, w2_sb = [], [], []
            for k in range(KD):
                t = persist.tile([P, din], f32r, name=f"wm{k}", tag=f"wm{k}")
                load_insts.append(nc.sync.dma_start(out=t, in_=w01_d[k * P:(k + 1) * P, 0:din]))
                wm_sb.append(t)
            for k in range(KD):
                t = persist.tile([P, dhid], f32r, name=f"w1_{k}", tag=f"w1_{k}")
                load_insts.append(nc.sync.dma_start(out=t, in_=w01_d[k * P:(k + 1) * P, din:din + dhid]))
                w1_sb.append(t)
            for k in range(KH):
                t = persist.tile([P, dout], f32r, name=f"w2_{k}", tag=f"w2_{k}")
                load_insts.append(nc.sync.dma_start(out=t, in_=w01_d[k * P:(k + 1) * P, din + dhid:dcat]))
                w2_sb.append(t)
            bias_sb = persist.tile([P, dcat], f32, name="bias_sb", tag="bias_sb")
            load_insts.append(nc.sync.dma_start(out=bias_sb, in_=bias_d[:, :]))
            bm_sb = bias_sb[:, 0:din]
            b1_sb = bias_sb[:, din:din + dhid]
            b2_sb = bias_sb[:, din + dhid:dcat]
            # whole x resident, 4 loads
            xb_all = persist.tile([P, T, din], f32r, name="xb_all", tag="xb_all")
            for i in range(T):
                load_insts.append(nc.sync.dma_start(
                    out=xb_all[:, i, :], in_=x_d[i * P:(i + 1) * P, :]))

            dinv = persist.tile([P, T], f32, name="dinv", tag="dinv")

            # adjT resident in SBUF: adjt[mp, j, nf] = adj[nf_global, j*128+mp]
            adjt = persist.tile([P, T, n], bf16, name="adjt", tag="adjt")

            xT = [persist.tile([P, n], f32r, name=f"xT{k}", tag=f"xT{k}")
                  for k in range(KD)]
            y1 = [persist.tile([P, din], bf16, name=f"y1_{i}", tag=f"y1_{i}")
                  for i in range(T)]

            # ---- all 16 xbar transposes back-to-back (nothing else between) -
            t_insts = []
            for i in range(T):
                isl = slice(i * P, (i + 1) * P)
                ti = nc.sync.dma_start_transpose(adjt[:, :, isl], adjb_d[isl, :])
                if i == 0:
                    for li in load_insts:
                        tile.add_dep_helper(ti.ins, li.ins, sync=True,
                                            reason="xbar waits all loads complete")
                else:
                    # the xbar engine is stateful: concurrent in-flight
                    # transposes interleave packets and scramble rows
                    tile.add_dep_helper(ti.ins, t_insts[i - 1].ins, sync=True,
                                        reason="serialize xbar transposes")
                t_insts.append(ti)

            # ---- per chunk: d/dinv, xT, Wm, y1 (PE chases the transposes) --
            for c in range(NCH):
                tiles = range(c * TC_, (c + 1) * TC_)
                # d for output rows of this chunk: ones^T-product over adjT
                csl = slice(c * CH, (c + 1) * CH)
                pdr = ps.tile([1, CH], f32, name="pdr", tag="pdrow", bufs=2)
                for j in range(T):
                    mm = nc.tensor.matmul(pdr, lhsT=onesb, rhs=adjt[:, j, csl],
                                          start=(j == 0), stop=(j == T - 1))
                    if j == 0:
                        for i in tiles:
                            tile.add_dep_helper(mm.ins, t_insts[i].ins, sync=True,
                                                reason="d-MM waits chunk transposes")
                drow = small.tile([1, CH], f32, name="drow", tag="drow")
                nc.vector.tensor_copy(drow, pdr)
                for q, i in enumerate(tiles):
                    pcol = ps.tile([P, 1], f32, name="pcol", tag="pdrow", bufs=2)
                    nc.tensor.matmul(pcol, lhsT=drow[0:1, q * P:(q + 1) * P],
                                     rhs=onef, start=True, stop=True)
                    srt = small.tile([P, 1], f32, name="srt", tag="srt")
                    nc.scalar.activation(srt, pcol, AF.Sqrt, bias=1.0)
                    nc.vector.reciprocal(dinv[:, i:i + 1], srt)

                for i in tiles:
                    isl = slice(i * P, (i + 1) * P)
                    # xT via PE transpose (f32r identity matmul)
                    for k in range(KD):
                        ptx = ps.tile([P, P], f32, name="ptx", tag="ptmp", bufs=4)
                        nc.tensor.matmul(ptx, lhsT=xb_all[:, i, k * P:(k + 1) * P],
                                         rhs=eyer, start=True, stop=True)
                        if (i + k) % 2 == 0:
                            nc.vector.tensor_copy(xT[k][:, isl], ptx)
                        else:
                            nc.scalar.copy(xT[k][:, isl], ptx)
                    # mv = relu(x@Wm + bm); y1 = bf16(dinv * mv)
                    pm = ps.tile([P, din], f32, name="pm", tag="pw", bufs=2)
                    for k in range(KD):
                        nc.tensor.matmul(pm, lhsT=xT[k][:, isl], rhs=wm_sb[k],
                                         start=(k == 0), stop=(k == KD - 1))
                    pre = small.tile([P, din], f32, name="pre", tag="pre")
                    nc.vector.tensor_add(pre, pm, bm_sb)
                    mvt = small.tile([P, din], f32, name="mvt", tag="mvt", bufs=16)
                    nc.scalar.activation(mvt, pre, AF.Relu)
                    nc.sync.dma_start(out=mv_d[isl, :], in_=mvt)
                    nc.vector.tensor_scalar(out=y1[i], in0=pre,
                                            scalar1=dinv[:, i:i + 1], scalar2=0.0,
                                            op0=OP.mult, op1=OP.max)

            # ---- A-products (bf16): tdst[db][:, c] = (adj @ y + y).T -------
            def a_product(ysrc, kb, tdst):
                for c in range(NCH):
                    c0 = c * CH
                    for db in range(kb):
                        pa = ps.tile([P, CH], f32, name="pa", tag="ptmp", bufs=4)
                        mms = []
                        for m in range(T):
                            mms.append((ysrc[m][:, db * P:(db + 1) * P],
                                        adjt[:, m, c0:c0 + CH], pa[:, :]))
                            if c0 <= m * P < c0 + CH:
                                off = m * P - c0
                                mms.append((ysrc[m][:, db * P:(db + 1) * P],
                                            eyeb, pa[:, off:off + P]))
                        for q, (l, r, o) in enumerate(mms):
                            mm = nc.tensor.matmul(o, lhsT=l, rhs=r, start=(q == 0),
                                                  stop=(q == len(mms) - 1))
                            if q == 0:
                                for it in range(c * TC_, (c + 1) * TC_):
                                    tile.add_dep_helper(
                                        mm.ins, t_insts[it].ins, sync=True,
                                        reason="A-product waits chunk transposes")
                        if (c + db) % 2 == 0:
                            nc.vector.tensor_copy(tdst[db][:, c0:c0 + CH], pa)
                        else:
                            nc.scalar.copy(tdst[db][:, c0:c0 + CH], pa)

            t1T = [feat.tile([P, n], f32r, name=f"t1T{k}", tag="featbig")
                   for k in range(KD)]
            a_product(y1, KD, t1T)

            # ---- hid = relu(dinv*(t1@W1) + b1); y2 = bf16(dinv * hid) ------
            y2 = [persist.tile([P, dhid], bf16, name=f"y2_{i}", tag=f"y2_{i}")
                  for i in range(T)]
            for i in range(T):
                isl = slice(i * P, (i + 1) * P)
                ph = ps.tile([P, dhid], f32, name="ph", tag="pw", bufs=2)
                for k in range(KD):
                    nc.tensor.matmul(ph, lhsT=t1T[k][:, isl], rhs=w1_sb[k],
                                     start=(k == 0), stop=(k == KD - 1))
                pre1 = small.tile([P, dhid], f32, name="pre1", tag="pre")
                nc.vector.scalar_tensor_tensor(pre1, ph, dinv[:, i:i + 1], b1_sb,
                                               op0=OP.mult, op1=OP.add)
                hidt = small.tile([P, dhid], f32, name="hidt", tag="hot")
                nc.scalar.activation(hidt, pre1, AF.Relu)
                nc.sync.dma_start(out=hid_d[isl, :], in_=hidt)
                nc.vector.tensor_scalar(out=y2[i], in0=pre1, scalar1=dinv[:, i:i + 1],
                                        scalar2=0.0, op0=OP.mult, op1=OP.max)

            if dbg:
                for qj, j in enumerate([0, 5, 8, 15]):
                    nc.sync.dma_start(out=adjt_dbg[qj], in_=adjt[:, j, :])
                for i in range(T):
                    yc = small.tile([P, din], f32, name="yc", tag="pre")
                    nc.vector.tensor_copy(yc, y1[i])
                    nc.sync.dma_start(out=y1_dbg[i], in_=yc)
                for k in range(KD):
                    tc_dbg = persist.tile([P, n], f32, name=f"tc_dbg{k}", tag=f"tcd{k}")
                    nc.vector.tensor_copy(tc_dbg, t1T[k])
                    nc.sync.dma_start(out=t1_dbg[k], in_=tc_dbg)
                nc.sync.dma_start(out=dinv_dbg[:, :], in_=dinv)

            t2T = [feat.tile([P, n], f32r, name=f"t2T{k}", tag="featbig")
                   for k in range(KH)]
            a_product(y2, KH, t2T)

            # ---- out = relu(dinv*(t2@W2) + b2) -----------------------------
            for i in range(T):
                isl = slice(i * P, (i + 1) * P)
                po = ps.tile([P, dout], f32, name="po", tag="pw", bufs=2)
                for k in range(KH):
                    nc.tensor.matmul(po, lhsT=t2T[k][:, isl], rhs=w2_sb[k],
                                     start=(k == 0), stop=(k == KH - 1))
                pre2 = small.tile([P, dout], f32, name="pre2", tag="pre")
                nc.vector.scalar_tensor_tensor(pre2, po, dinv[:, i:i + 1], b2_sb,
                                               op0=OP.mult, op1=OP.add)
                outt = small.tile([P, dout], f32, name="outt", tag="hot")
                nc.scalar.activation(outt, pre2, AF.Relu)
                nc.sync.dma_start(out=out_d[isl, :], in_=outt)

    nc.compile()
    return nc


VARIANT = "bf16"
_NC_CACHE = None


def _get_nc():
    global _NC_CACHE
    if _NC_CACHE is None:
        _NC_CACHE = _build_module_bf16()
    return _NC_CACHE


def _make_in_maps(adj, x, w_mean, b_mean, w1, b1, w2, b2):
    import ml_dtypes

    adj = np.asarray(adj, dtype=np.float32)
    x = np.asarray(x, dtype=np.float32)
    wm = np.ascontiguousarray(np.asarray(w_mean, dtype=np.float32))
    w1 = np.ascontiguousarray(np.asarray(w1, dtype=np.float32))
    w2 = np.ascontiguousarray(np.asarray(w2, dtype=np.float32))
    bm = np.ascontiguousarray(np.broadcast_to(np.asarray(b_mean, np.float32), (P, IN_DIM)))
    b1b = np.ascontiguousarray(np.broadcast_to(np.asarray(b1, np.float32), (P, HID_DIM)))
    b2b = np.ascontiguousarray(np.broadcast_to(np.asarray(b2, np.float32), (P, OUT_DIM)))
    w01 = np.ascontiguousarray(np.concatenate([wm, w1, w2], axis=1))
    bias = np.ascontiguousarray(np.concatenate([bm, b1b, b2b], axis=1))
    eyeb = np.eye(P, dtype=ml_dtypes.bfloat16)
    onesb = np.ones((P, 2), dtype=ml_dtypes.bfloat16)
    eyef = np.eye(P, dtype=np.float32)
    onef = np.ones((1, 1), dtype=np.float32)
    return [
        dict(adjb=np.ascontiguousarray(adj[b].astype(ml_dtypes.bfloat16)),
             x=np.ascontiguousarray(x[b]),
             w01=w01, bias=bias, eyeb=eyeb, onesb=onesb, eyef=eyef, onef=onef)
        for b in range(B)
    ]


def kernel(adj, gcn_inputs, w_mean, b_mean, w1, b1, w2, b2):
    from concourse.bass_utils import run_bass_kernel_spmd

    nc = _get_nc()
    in_maps = _make_in_maps(adj, gcn_inputs, w_mean, b_mean, w1, b1, w2, b2)
    res = run_bass_kernel_spmd(nc, in_maps, core_ids=list(range(B)))
    mv = np.stack([res.results[b]["mv"] for b in range(B)])
    hid = np.stack([res.results[b]["hid"] for b in range(B)])
    out = np.stack([res.results[b]["out"] for b in range(B)])
    x = np.asarray(gcn_inputs, dtype=np.float32)
    return ((x, mv, hid, out), ())


# revision 38
# speedup vs baseline: 2.9713x; 1.2145x over previous
"""GCN (CGCN) forward kernel for Trainium2, data-parallel over batch on 8 NeuronCores.

Per core (one batch sample):
  d      = adj.sum(-1) + 1 ;  dinv = d**-0.5
  mv     = relu(x @ Wm + bm)
  t1     = adj @ (dinv*mv) + (dinv*mv)          # A @ Dinv @ mv  with A = adj + I
  hid    = relu(dinv*(t1 @ W1) + b1)
  t2     = adj @ (dinv*hid) + (dinv*hid)
  out    = relu(dinv*(t2 @ W2) + b2)

adj is shipped to the device in bf16 and transposed by the DMA xbar engine
straight from DRAM into a resident SBUF tensor (the PE contracts over the
partition axis, so adj@v needs adj^T tiles).  Row sums come from PE
ones-products over the transposed tiles, so adj is read from HBM exactly once.
All matmuls run in bf16 with fp32 PSUM accumulation; everything else
(normalization, biases, relu, outputs) stays fp32.
"""

import numpy as np

B, N, IN_DIM, HID_DIM, OUT_DIM = 8, 2048, 256, 256, 128
P = 128


def _build_module_bf16(n=N, din=IN_DIM, dhid=HID_DIM, dout=OUT_DIM, dbg=False):
    from contextlib import ExitStack

    import concourse.mybir as mybir
    import concourse.tile as tile
    from concourse import bacc

    f32 = mybir.dt.float32
    f32r = mybir.dt.float32r
    bf16 = mybir.dt.bfloat16
    OP = mybir.AluOpType
    AF = mybir.ActivationFunctionType

    T = n // P
    KD = din // P
    KH = dhid // P
    CH = min(512, n)
    NCH = n // CH
    TC_ = CH // P  # stream tiles per chunk

    nc = bacc.Bacc(None, target_bir_lowering=False)

    dcat = din + dhid + dout
    adjb_d = nc.declare_dram_parameter("adjb", [n, n], bf16, isOutput=False)
    x_d = nc.declare_dram_parameter("x", [n, din], f32r, isOutput=False)
    w01_d = nc.declare_dram_parameter("w01", [din, dcat], f32r, isOutput=False)
    bias_d = nc.declare_dram_parameter("bias", [P, dcat], f32, isOutput=False)
    eyeb_d = nc.declare_dram_parameter("eyeb", [P, P], bf16, isOutput=False)
    onesb_d = nc.declare_dram_parameter("onesb", [P, 2], bf16, isOutput=False)
    eyef_d = nc.declare_dram_parameter("eyef", [P, P], f32r, isOutput=False)
    onef_d = nc.declare_dram_parameter("onef", [1, 1], f32, isOutput=False)
    mv_d = nc.declare_dram_parameter("mv", [n, din], f32, isOutput=True)
    hid_d = nc.declare_dram_parameter("hid", [n, dhid], f32, isOutput=True)
    out_d = nc.declare_dram_parameter("out", [n, dout], f32, isOutput=True)
    if dbg:
        y1_dbg = nc.declare_dram_parameter("y1_dbg", [T, P, din], f32, isOutput=True)
        t1_dbg = nc.declare_dram_parameter("t1_dbg", [KD, P, n], f32, isOutput=True)
        dinv_dbg = nc.declare_dram_parameter("dinv_dbg", [P, T], f32, isOutput=True)
        adjt_dbg = nc.declare_dram_parameter("adjt_dbg", [4, P, n], bf16, isOutput=True)

    with tile.TileContext(nc) as tc:
        with ExitStack() as ctx:
            persist = ctx.enter_context(tc.tile_pool(name="persist", bufs=1))
            small = ctx.enter_context(tc.tile_pool(name="small", bufs=4))
            feat = ctx.enter_context(tc.tile_pool(name="feat", bufs=2))
            ps = ctx.enter_context(tc.tile_pool(name="ps", bufs=2, space="PSUM"))

            # ---- all loads on the SP ring, BEFORE the xbar transposes ----
            load_insts = []
            eyeb = persist.tile([P, P], bf16, name="eyeb", tag="eyeb")
            load_insts.append(nc.sync.dma_start(out=eyeb, in_=eyeb_d[:, :]))
            onesb2 = persist.tile([P, 2], bf16, name="onesb2", tag="onesb2")
            load_insts.append(nc.sync.dma_start(out=onesb2, in_=onesb_d[:, :]))
            onesb = onesb2[:, 0:1]
            eyer = persist.tile([P, P], f32r, name="eyer", tag="eyer")
            load_insts.append(nc.sync.dma_start(out=eyer, in_=eyef_d[:, :]))
            onef = persist.tile([1, 1], f32, name="onef", tag="onef")
            load_insts.append(nc.sync.dma_start(out=onef, in_=onef_d[:, :]))
            wm_sb, w1_sb, w2_sb = [], [], []
            for k in range(KD):
                t = persist.tile([P, din], f32r, name=f"wm{k}", tag=f"wm{k}")
                load_insts.append(nc.sync.dma_start(out=t, in_=w01_d[k * P:(k + 1) * P, 0:din]))
                wm_sb.append(t)
            for k in range(KD):
                t = persist.tile([P, dhid], f32r, name=f"w1_{k}", tag=f"w1_{k}")
                load_insts.append(nc.sync.dma_start(out=t, in_=w01_d[k * P:(k + 1) * P, din:din + dhid]))
                w1_sb.append(t)
            for k in range(KH):
                t = persist.tile([P, dout], f32r, name=f"w2_{k}", tag=f"w2_{k}")
                load_insts.append(nc.sync.dma_start(out=t, in_=w01_d[k * P:(k + 1) * P, din + dhid:dcat]))
                w2_sb.append(t)
            bias_sb = persist.tile([P, dcat], f32, name="bias_sb", tag="bias_sb")
            load_insts.append(nc.sync.dma_start(out=bias_sb, in_=bias_d[:, :]))
            bm_sb = bias_sb[:, 0:din]
            b1_sb = bias_sb[:, din:din + dhid]
            b2_sb = bias_sb[:, din + dhid:dcat]
            # whole x resident, 4 loads
            xb_all = persist.tile([P, T, din], f32r, name="xb_all", tag="xb_all")
            for i in range(T):
                load_insts.append(nc.sync.dma_start(
                    out=xb_all[:, i, :], in_=x_d[i * P:(i + 1) * P, :]))

            dinv = persist.tile([P, T], f32, name="dinv", tag="dinv")

            # adjT resident in SBUF: adjt[mp, j, nf] = adj[nf_global, j*128+mp]
            adjt = persist.tile([P, T, n], bf16, name="adjt", tag="adjt")

            xT = [persist.tile([P, n], f32r, name=f"xT{k}", tag=f"xT{k}")
                  for k in range(KD)]
            y1 = [persist.tile([P, din], bf16, name=f"y1_{i}", tag=f"y1_{i}")
                  for i in range(T)]

            # ---- xbar transposes, one per 512-column chunk, strictly
            # serialized: the xbar engine is stateful and concurrent in-flight
            # transposes (or transposes overlapping regular DMAs) scramble rows.
            t_insts = []
            for c in range(NCH):
                csl = slice(c * CH, (c + 1) * CH)
                ti = nc.sync.dma_start_transpose(adjt[:, :, csl], adjb_d[csl, :])
                if c == 0:
                    for li in load_insts:
                        tile.add_dep_helper(ti.ins, li.ins, sync=True,
                                            reason="xbar waits all loads complete")
                else:
                    tile.add_dep_helper(ti.ins, t_insts[c - 1].ins, sync=True,
                                        reason="serialize xbar transposes")
                t_insts.append(ti)

            # ---- per chunk: d/dinv, xT, Wm, y1 (PE chases the transposes) --
            for c in range(NCH):
                tiles = range(c * TC_, (c + 1) * TC_)
                # d for output rows of this chunk: ones^T-product over adjT
                csl = slice(c * CH, (c + 1) * CH)
                pdr = ps.tile([1, CH], f32, name="pdr", tag="pdrow", bufs=2)
                for j in range(T):
                    mm = nc.tensor.matmul(pdr, lhsT=onesb, rhs=adjt[:, j, csl],
                                          start=(j == 0), stop=(j == T - 1))
                    if j == 0:
                        tile.add_dep_helper(mm.ins, t_insts[c].ins, sync=True,
                                            reason="d-MM waits chunk transpose")
                drow = small.tile([1, CH], f32, name="drow", tag="drow")
                nc.vector.tensor_copy(drow, pdr)
                for q, i in enumerate(tiles):
                    pcol = ps.tile([P, 1], f32, name="pcol", tag="pdrow", bufs=2)
                    nc.tensor.matmul(pcol, lhsT=drow[0:1, q * P:(q + 1) * P],
                                     rhs=onef, start=True, stop=True)
                    srt = small.tile([P, 1], f32, name="srt", tag="srt")
                    nc.scalar.activation(srt, pcol, AF.Sqrt, bias=1.0)
                    nc.vector.reciprocal(dinv[:, i:i + 1], srt)

                for i in tiles:
                    isl = slice(i * P, (i + 1) * P)
                    # xT via PE transpose (f32r identity matmul)
                    for k in range(KD):
                        ptx = ps.tile([P, P], f32, name="ptx", tag="ptmp", bufs=4)
                        nc.tensor.matmul(ptx, lhsT=xb_all[:, i, k * P:(k + 1) * P],
                                         rhs=eyer, start=True, stop=True)
                        if (i + k) % 2 == 0:
                            nc.vector.tensor_copy(xT[k][:, isl], ptx)
                        else:
                            nc.scalar.copy(xT[k][:, isl], ptx)
                    # mv = relu(x@Wm + bm); y1 = bf16(dinv * mv)
                    pm = ps.tile([P, din], f32, name="pm", tag="pw", bufs=2)
                    for k in range(KD):
                        nc.tensor.matmul(pm, lhsT=xT[k][:, isl], rhs=wm_sb[k],
                                         start=(k == 0), stop=(k == KD - 1))
                    pre = small.tile([P, din], f32, name="pre", tag="pre")
                    nc.vector.tensor_add(pre, pm, bm_sb)
                    mvt = small.tile([P, din], f32, name="mvt", tag="mvt", bufs=16)
                    nc.scalar.activation(mvt, pre, AF.Relu)
                    nc.sync.dma_start(out=mv_d[isl, :], in_=mvt)
                    nc.vector.tensor_scalar(out=y1[i], in0=pre,
                                            scalar1=dinv[:, i:i + 1], scalar2=0.0,
                                            op0=OP.mult, op1=OP.max)

            # ---- A-products (bf16): tdst[db][:, c] = (adj @ y + y).T -------
            def a_product(ysrc, kb, tdst):
                for c in range(NCH):
                    c0 = c * CH
                    for db in range(kb):
                        pa = ps.tile([P, CH], f32, name="pa", tag="ptmp", bufs=4)
                        mms = []
                        for m in range(T):
                            mms.append((ysrc[m][:, db * P:(db + 1) * P],
                                        adjt[:, m, c0:c0 + CH], pa[:, :]))
                            if c0 <= m * P < c0 + CH:
                                off = m * P - c0
                                mms.append((ysrc[m][:, db * P:(db + 1) * P],
                                            eyeb, pa[:, off:off + P]))
                        for q, (l, r, o) in enumerate(mms):
                            mm = nc.tensor.matmul(o, lhsT=l, rhs=r, start=(q == 0),
                                                  stop=(q == len(mms) - 1))
                            if q == 0:
                                tile.add_dep_helper(
                                    mm.ins, t_insts[c].ins, sync=True,
                                    reason="A-product waits chunk transpose")
                        if (c + db) % 2 == 0:
                            nc.vector.tensor_copy(tdst[db][:, c0:c0 + CH], pa)
                        else:
                            nc.scalar.copy(tdst[db][:, c0:c0 + CH], pa)

            t1T = [feat.tile([P, n], f32r, name=f"t1T{k}", tag="featbig")
                   for k in range(KD)]
            a_product(y1, KD, t1T)

            # ---- hid = relu(dinv*(t1@W1) + b1); y2 = bf16(dinv * hid) ------
            y2 = [persist.tile([P, dhid], bf16, name=f"y2_{i}", tag=f"y2_{i}")
                  for i in range(T)]
            for i in range(T):
                isl = slice(i * P, (i + 1) * P)
                ph = ps.tile([P, dhid], f32, name="ph", tag="pw", bufs=2)
                for k in range(KD):
                    nc.tensor.matmul(ph, lhsT=t1T[k][:, isl], rhs=w1_sb[k],
                                     start=(k == 0), stop=(k == KD - 1))
                pre1 = small.tile([P, dhid], f32, name="pre1", tag="pre")
                nc.vector.scalar_tensor_tensor(pre1, ph, dinv[:, i:i + 1], b1_sb,
                                               op0=OP.mult, op1=OP.add)
                hidt = small.tile([P, dhid], f32, name="hidt", tag="hot")
                nc.scalar.activation(hidt, pre1, AF.Relu)
                nc.sync.dma_start(out=hid_d[isl, :], in_=hidt)
                nc.vector.tensor_scalar(out=y2[i], in0=pre1, scalar1=dinv[:, i:i + 1],
                                        scalar2=0.0, op0=OP.mult, op1=OP.max)

            if dbg:
                for qj, j in enumerate([0, 5, 8, 15]):
                    nc.sync.dma_start(out=adjt_dbg[qj], in_=adjt[:, j, :])
                for i in range(T):
                    yc = small.tile([P, din], f32, name="yc", tag="pre")
                    nc.vector.tensor_copy(yc, y1[i])
                    nc.sync.dma_start(out=y1_dbg[i], in_=yc)
                for k in range(KD):
                    tc_dbg = persist.tile([P, n], f32, name=f"tc_dbg{k}", tag=f"tcd{k}")
                    nc.vector.tensor_copy(tc_dbg, t1T[k])
                    nc.sync.dma_start(out=t1_dbg[k], in_=tc_dbg)
                nc.sync.dma_start(out=dinv_dbg[:, :], in_=dinv)

            t2T = [feat.tile([P, n], f32r, name=f"t2T{k}", tag="featbig")
                   for k in range(KH)]
            a_product(y2, KH, t2T)

            # ---- out = relu(dinv*(t2@W2) + b2) -----------------------------
            for i in range(T):
                isl = slice(i * P, (i + 1) * P)
                po = ps.tile([P, dout], f32, name="po", tag="pw", bufs=2)
                for k in range(KH):
                    nc.tensor.matmul(po, lhsT=t2T[k][:, isl], rhs=w2_sb[k],
                                     start=(k == 0), stop=(k == KH - 1))
                pre2 = small.tile([P, dout], f32, name="pre2", tag="pre")
                nc.vector.scalar_tensor_tensor(pre2, po, dinv[:, i:i + 1], b2_sb,
                                               op0=OP.mult, op1=OP.add)
                outt = small.tile([P, dout], f32, name="outt", tag="hot")
                nc.scalar.activation(outt, pre2, AF.Relu)
                nc.sync.dma_start(out=out_d[isl, :], in_=outt)

    nc.compile()
    return nc


VARIANT = "bf16"
_NC_CACHE = None


def _get_nc():
    global _NC_CACHE
    if _NC_CACHE is None:
        _NC_CACHE = _build_module_bf16()
    return _NC_CACHE


def _make_in_maps(adj, x, w_mean, b_mean, w1, b1, w2, b2):
    import ml_dtypes

    adj = np.asarray(adj, dtype=np.float32)
    x = np.asarray(x, dtype=np.float32)
    wm = np.ascontiguousarray(np.asarray(w_mean, dtype=np.float32))
    w1 = np.ascontiguousarray(np.asarray(w1, dtype=np.float32))
    w2 = np.ascontiguousarray(np.asarray(w2, dtype=np.float32))
    bm = np.ascontiguousarray(np.broadcast_to(np.asarray(b_mean, np.float32), (P, IN_DIM)))
    b1b = np.ascontiguousarray(np.broadcast_to(np.asarray(b1, np.float32), (P, HID_DIM)))
    b2b = np.ascontiguousarray(np.broadcast_to(np.asarray(b2, np.float32), (P, OUT_DIM)))
    w01 = np.ascontiguousarray(np.concatenate([wm, w1, w2], axis=1))
    bias = np.ascontiguousarray(np.concatenate([bm, b1b, b2b], axis=1))
    eyeb = np.eye(P, dtype=ml_dtypes.bfloat16)
    onesb = np.ones((P, 2), dtype=ml_dtypes.bfloat16)
    eyef = np.eye(P, dtype=np.float32)
    onef = np.ones((1, 1), dtype=np.float32)
    return [
        dict(adjb=np.ascontiguousarray(adj[b].astype(ml_dtypes.bfloat16)),
             x=np.ascontiguousarray(x[b]),
             w01=w01, bias=bias, eyeb=eyeb, onesb=onesb, eyef=eyef, onef=onef)
        for b in range(B)
    ]


def kernel(adj, gcn_inputs, w_mean, b_mean, w1, b1, w2, b2):
    from concourse.bass_utils import run_bass_kernel_spmd

    nc = _get_nc()
    in_maps = _make_in_maps(adj, gcn_inputs, w_mean, b_mean, w1, b1, w2, b2)
    res = run_bass_kernel_spmd(nc, in_maps, core_ids=list(range(B)))
    mv = np.stack([res.results[b]["mv"] for b in range(B)])
    hid = np.stack([res.results[b]["hid"] for b in range(B)])
    out = np.stack([res.results[b]["out"] for b in range(B)])
    x = np.asarray(gcn_inputs, dtype=np.float32)
    return ((x, mv, hid, out), ())


# revision 40
# speedup vs baseline: 3.0529x; 1.0275x over previous
"""GCN (CGCN) forward kernel for Trainium2, data-parallel over batch on 8 NeuronCores.

Per core (one batch sample):
  d      = adj.sum(-1) + 1 ;  dinv = d**-0.5
  mv     = relu(x @ Wm + bm)
  t1     = adj @ (dinv*mv) + (dinv*mv)          # A @ Dinv @ mv  with A = adj + I
  hid    = relu(dinv*(t1 @ W1) + b1)
  t2     = adj @ (dinv*hid) + (dinv*hid)
  out    = relu(dinv*(t2 @ W2) + b2)

adj is shipped to the device in bf16 and transposed by the DMA xbar engine
straight from DRAM into a resident SBUF tensor (the PE contracts over the
partition axis, so adj@v needs adj^T tiles).  Row sums come from PE
ones-products over the transposed tiles, so adj is read from HBM exactly once.
All matmuls run in bf16 with fp32 PSUM accumulation; everything else
(normalization, biases, relu, outputs) stays fp32.
"""

import numpy as np

B, N, IN_DIM, HID_DIM, OUT_DIM = 8, 2048, 256, 256, 128
P = 128


def _build_module_bf16(n=N, din=IN_DIM, dhid=HID_DIM, dout=OUT_DIM, dbg=False):
    from contextlib import ExitStack

    import concourse.mybir as mybir
    import concourse.tile as tile
    from concourse import bacc

    f32 = mybir.dt.float32
    f32r = mybir.dt.float32r
    bf16 = mybir.dt.bfloat16
    OP = mybir.AluOpType
    AF = mybir.ActivationFunctionType

    T = n // P
    KD = din // P
    KH = dhid // P
    CH = min(512, n)
    NCH = n // CH
    TC_ = CH // P  # stream tiles per chunk

    nc = bacc.Bacc(None, target_bir_lowering=False)

    dcat = din + dhid + dout
    adjb_d = nc.declare_dram_parameter("adjb", [n, n], bf16, isOutput=False)
    x_d = nc.declare_dram_parameter("x", [n, din], f32r, isOutput=False)
    w01_d = nc.declare_dram_parameter("w01", [din, dcat], f32r, isOutput=False)
    bias_d = nc.declare_dram_parameter("bias", [P, dcat], f32, isOutput=False)
    eyeb_d = nc.declare_dram_parameter("eyeb", [P, P], bf16, isOutput=False)
    onesb_d = nc.declare_dram_parameter("onesb", [P, 2], bf16, isOutput=False)
    eyef_d = nc.declare_dram_parameter("eyef", [P, P], f32r, isOutput=False)
    onef_d = nc.declare_dram_parameter("onef", [1, 1], f32, isOutput=False)
    mv_d = nc.declare_dram_parameter("mv", [n, din], f32, isOutput=True)
    hid_d = nc.declare_dram_parameter("hid", [n, dhid], f32, isOutput=True)
    out_d = nc.declare_dram_parameter("out", [n, dout], f32, isOutput=True)
    if dbg:
        y1_dbg = nc.declare_dram_parameter("y1_dbg", [T, P, din], f32, isOutput=True)
        t1_dbg = nc.declare_dram_parameter("t1_dbg", [KD, P, n], f32, isOutput=True)
        dinv_dbg = nc.declare_dram_parameter("dinv_dbg", [P, T], f32, isOutput=True)
        adjt_dbg = nc.declare_dram_parameter("adjt_dbg", [4, P, n], bf16, isOutput=True)

    with tile.TileContext(nc) as tc:
        with ExitStack() as ctx:
            persist = ctx.enter_context(tc.tile_pool(name="persist", bufs=1))
            small = ctx.enter_context(tc.tile_pool(name="small", bufs=4))
            feat = ctx.enter_context(tc.tile_pool(name="feat", bufs=2))
            ps = ctx.enter_context(tc.tile_pool(name="ps", bufs=2, space="PSUM"))

            # ---- all loads on the SP ring, BEFORE the xbar transposes ----
            load_insts = []
            eyeb = persist.tile([P, P], bf16, name="eyeb", tag="eyeb")
            load_insts.append(nc.sync.dma_start(out=eyeb, in_=eyeb_d[:, :]))
            onesb2 = persist.tile([P, 2], bf16, name="onesb2", tag="onesb2")
            load_insts.append(nc.sync.dma_start(out=onesb2, in_=onesb_d[:, :]))
            onesb = onesb2[:, 0:1]
            eyer = persist.tile([P, P], f32r, name="eyer", tag="eyer")
            load_insts.append(nc.sync.dma_start(out=eyer, in_=eyef_d[:, :]))
            onef = persist.tile([1, 1], f32, name="onef", tag="onef")
            load_insts.append(nc.sync.dma_start(out=onef, in_=onef_d[:, :]))
            wm_sb, w1_sb, w2_sb = [], [], []
            for k in range(KD):
                t = persist.tile([P, din], f32r, name=f"wm{k}", tag=f"wm{k}")
                load_insts.append(nc.sync.dma_start(out=t, in_=w01_d[k * P:(k + 1) * P, 0:din]))
                wm_sb.append(t)
            for k in range(KD):
                t = persist.tile([P, dhid], f32r, name=f"w1_{k}", tag=f"w1_{k}")
                load_insts.append(nc.sync.dma_start(out=t, in_=w01_d[k * P:(k + 1) * P, din:din + dhid]))
                w1_sb.append(t)
            for k in range(KH):
                t = persist.tile([P, dout], f32r, name=f"w2_{k}", tag=f"w2_{k}")
                load_insts.append(nc.sync.dma_start(out=t, in_=w01_d[k * P:(k + 1) * P, din + dhid:dcat]))
                w2_sb.append(t)
            bias_sb = persist.tile([P, dcat], f32, name="bias_sb", tag="bias_sb")
            load_insts.append(nc.sync.dma_start(out=bias_sb, in_=bias_d[:, :]))
            bm_sb = bias_sb[:, 0:din]
            b1_sb = bias_sb[:, din:din + dhid]
            b2_sb = bias_sb[:, din + dhid:dcat]
            # whole x resident, 4 loads
            xb_all = persist.tile([P, T, din], f32r, name="xb_all", tag="xb_all")
            for cq in range((T + 3) // 4):
                i0 = cq * 4
                g = min(4, T - i0)
                load_insts.append(nc.sync.dma_start(
                    out=xb_all[:, i0:i0 + g, :],
                    in_=x_d[i0 * P:(i0 + g) * P, :].rearrange(
                        "(i p) f -> p i f", p=P)))

            dinv = persist.tile([P, T], f32, name="dinv", tag="dinv")

            # adjT resident in SBUF: adjt[mp, j, nf] = adj[nf_global, j*128+mp]
            adjt = persist.tile([P, T, n], bf16, name="adjt", tag="adjt")

            xT = [persist.tile([P, n], f32r, name=f"xT{k}", tag=f"xT{k}")
                  for k in range(KD)]
            y1 = [persist.tile([P, din], bf16, name=f"y1_{i}", tag=f"y1_{i}")
                  for i in range(T)]

            # ---- xbar transposes, one per 512-column chunk, strictly
            # serialized: the xbar engine is stateful and concurrent in-flight
            # transposes (or transposes overlapping regular DMAs) scramble rows.
            t_insts = []
            for c in range(NCH):
                csl = slice(c * CH, (c + 1) * CH)
                ti = nc.sync.dma_start_transpose(adjt[:, :, csl], adjb_d[csl, :])
                if c == 0:
                    for li in load_insts:
                        tile.add_dep_helper(ti.ins, li.ins, sync=True,
                                            reason="xbar waits all loads complete")
                else:
                    tile.add_dep_helper(ti.ins, t_insts[c - 1].ins, sync=True,
                                        reason="serialize xbar transposes")
                t_insts.append(ti)

            # ---- per chunk: d/dinv, xT, Wm, y1 (PE chases the transposes) --
            for c in range(NCH):
                tiles = range(c * TC_, (c + 1) * TC_)
                # d for output rows of this chunk: ones^T-product over adjT
                csl = slice(c * CH, (c + 1) * CH)
                pdr = ps.tile([1, CH], f32, name="pdr", tag="pdrow", bufs=2)
                for j in range(T):
                    mm = nc.tensor.matmul(pdr, lhsT=onesb, rhs=adjt[:, j, csl],
                                          start=(j == 0), stop=(j == T - 1))
                    if j == 0:
                        tile.add_dep_helper(mm.ins, t_insts[c].ins, sync=True,
                                            reason="d-MM waits chunk transpose")
                drow = small.tile([1, CH], f32, name="drow", tag="drow")
                nc.vector.tensor_copy(drow, pdr)
                for q, i in enumerate(tiles):
                    pcol = ps.tile([P, 1], f32, name="pcol", tag="pdrow", bufs=2)
                    nc.tensor.matmul(pcol, lhsT=drow[0:1, q * P:(q + 1) * P],
                                     rhs=onef, start=True, stop=True)
                    srt = small.tile([P, 1], f32, name="srt", tag="srt")
                    nc.scalar.activation(srt, pcol, AF.Sqrt, bias=1.0)
                    nc.vector.reciprocal(dinv[:, i:i + 1], srt)

                for i in tiles:
                    isl = slice(i * P, (i + 1) * P)
                    # xT via PE transpose (f32r identity matmul)
                    for k in range(KD):
                        ptx = ps.tile([P, P], f32, name="ptx", tag="ptmp", bufs=4)
                        nc.tensor.matmul(ptx, lhsT=xb_all[:, i, k * P:(k + 1) * P],
                                         rhs=eyer, start=True, stop=True)
                        if (i + k) % 2 == 0:
                            nc.vector.tensor_copy(xT[k][:, isl], ptx)
                        else:
                            nc.scalar.copy(xT[k][:, isl], ptx)
                    # mv = relu(x@Wm + bm); y1 = bf16(dinv * mv)
                    pm = ps.tile([P, din], f32, name="pm", tag="pw", bufs=2)
                    for k in range(KD):
                        nc.tensor.matmul(pm, lhsT=xT[k][:, isl], rhs=wm_sb[k],
                                         start=(k == 0), stop=(k == KD - 1))
                    pre = small.tile([P, din], f32, name="pre", tag="pre")
                    nc.vector.tensor_add(pre, pm, bm_sb)
                    mvt = small.tile([P, din], f32, name="mvt", tag="mvt", bufs=16)
                    nc.scalar.activation(mvt, pre, AF.Relu)
                    nc.sync.dma_start(out=mv_d[isl, :], in_=mvt)
                    nc.vector.tensor_scalar(out=y1[i], in0=pre,
                                            scalar1=dinv[:, i:i + 1], scalar2=0.0,
                                            op0=OP.mult, op1=OP.max)

            # ---- A-products (bf16): tdst[db][:, c] = (adj @ y + y).T -------
            def a_product(ysrc, kb, tdst):
                for c in range(NCH):
                    c0 = c * CH
                    for db in range(kb):
                        pa = ps.tile([P, CH], f32, name="pa", tag="ptmp", bufs=4)
                        mms = []
                        for m in range(T):
                            mms.append((ysrc[m][:, db * P:(db + 1) * P],
                                        adjt[:, m, c0:c0 + CH], pa[:, :]))
                            if c0 <= m * P < c0 + CH:
                                off = m * P - c0
                                mms.append((ysrc[m][:, db * P:(db + 1) * P],
                                            eyeb, pa[:, off:off + P]))
                        for q, (l, r, o) in enumerate(mms):
                            mm = nc.tensor.matmul(o, lhsT=l, rhs=r, start=(q == 0),
                                                  stop=(q == len(mms) - 1))
                            if q == 0:
                                tile.add_dep_helper(
                                    mm.ins, t_insts[c].ins, sync=True,
                                    reason="A-product waits chunk transpose")
                        if (c + db) % 2 == 0:
                            nc.vector.tensor_copy(tdst[db][:, c0:c0 + CH], pa)
                        else:
                            nc.scalar.copy(tdst[db][:, c0:c0 + CH], pa)

            t1T = [feat.tile([P, n], f32r, name=f"t1T{k}", tag="featbig")
                   for k in range(KD)]
            a_product(y1, KD, t1T)

            # ---- hid = relu(dinv*(t1@W1) + b1); y2 = bf16(dinv * hid) ------
            y2 = [persist.tile([P, dhid], bf16, name=f"y2_{i}", tag=f"y2_{i}")
                  for i in range(T)]
            for i in range(T):
                isl = slice(i * P, (i + 1) * P)
                ph = ps.tile([P, dhid], f32, name="ph", tag="pw", bufs=2)
                for k in range(KD):
                    nc.tensor.matmul(ph, lhsT=t1T[k][:, isl], rhs=w1_sb[k],
                                     start=(k == 0), stop=(k == KD - 1))
                pre1 = small.tile([P, dhid], f32, name="pre1", tag="pre")
                nc.vector.scalar_tensor_tensor(pre1, ph, dinv[:, i:i + 1], b1_sb,
                                               op0=OP.mult, op1=OP.add)
                hidt = small.tile([P, dhid], f32, name="hidt", tag="hot")
                nc.scalar.activation(hidt, pre1, AF.Relu)
                nc.sync.dma_start(out=hid_d[isl, :], in_=hidt)
                nc.vector.tensor_scalar(out=y2[i], in0=pre1, scalar1=dinv[:, i:i + 1],
                                        scalar2=0.0, op0=OP.mult, op1=OP.max)

            if dbg:
                for qj, j in enumerate([0, 5, 8, 15]):
                    nc.sync.dma_start(out=adjt_dbg[qj], in_=adjt[:, j, :])
                for i in range(T):
                    yc = small.tile([P, din], f32, name="yc", tag="pre")
                    nc.vector.tensor_copy(yc, y1[i])
                    nc.sync.dma_start(out=y1_dbg[i], in_=yc)
                for k in range(KD):
                    tc_dbg = persist.tile([P, n], f32, name=f"tc_dbg{k}", tag=f"tcd{k}")
                    nc.vector.tensor_copy(tc_dbg, t1T[k])
                    nc.sync.dma_start(out=t1_dbg[k], in_=tc_dbg)
                nc.sync.dma_start(out=dinv_dbg[:, :], in_=dinv)

            t2T = [feat.tile([P, n], f32r, name=f"t2T{k}", tag="featbig")
                   for k in range(KH)]
            a_product(y2, KH, t2T)

            # ---- out = relu(dinv*(t2@W2) + b2), one batched store ----------
            out_acc = persist.tile([P, T, dout], f32, name="out_acc", tag="out_acc")
            for i in range(T):
                isl = slice(i * P, (i + 1) * P)
                po = ps.tile([P, dout], f32, name="po", tag="pw", bufs=2)
                for k in range(KH):
                    nc.tensor.matmul(po, lhsT=t2T[k][:, isl], rhs=w2_sb[k],
                                     start=(k == 0), stop=(k == KH - 1))
                pre2 = small.tile([P, dout], f32, name="pre2", tag="pre")
                nc.vector.scalar_tensor_tensor(pre2, po, dinv[:, i:i + 1], b2_sb,
                                               op0=OP.mult, op1=OP.add)
                nc.scalar.activation(out_acc[:, i, :], pre2, AF.Relu)
            nc.sync.dma_start(
                out=out_d.rearrange("(i p) f -> p i f", p=P), in_=out_acc)

    nc.compile()
    return nc


VARIANT = "bf16"
_NC_CACHE = None


def _get_nc():
    global _NC_CACHE
    if _NC_CACHE is None:
        _NC_CACHE = _build_module_bf16()
    return _NC_CACHE


def _make_in_maps(adj, x, w_mean, b_mean, w1, b1, w2, b2):
    import ml_dtypes

    adj = np.asarray(adj, dtype=np.float32)
    x = np.asarray(x, dtype=np.float32)
    wm = np.ascontiguousarray(np.asarray(w_mean, dtype=np.float32))
    w1 = np.ascontiguousarray(np.asarray(w1, dtype=np.float32))
    w2 = np.ascontiguousarray(np.asarray(w2, dtype=np.float32))
    bm = np.ascontiguousarray(np.broadcast_to(np.asarray(b_mean, np.float32), (P, IN_DIM)))
    b1b = np.ascontiguousarray(np.broadcast_to(np.asarray(b1, np.float32), (P, HID_DIM)))
    b2b = np.ascontiguousarray(np.broadcast_to(np.asarray(b2, np.float32), (P, OUT_DIM)))
    w01 = np.ascontiguousarray(np.concatenate([wm, w1, w2], axis=1))
    bias = np.ascontiguousarray(np.concatenate([bm, b1b, b2b], axis=1))
    eyeb = np.eye(P, dtype=ml_dtypes.bfloat16)
    onesb = np.ones((P, 2), dtype=ml_dtypes.bfloat16)
    eyef = np.eye(P, dtype=np.float32)
    onef = np.ones((1, 1), dtype=np.float32)
    return [
        dict(adjb=np.ascontiguousarray(adj[b].astype(ml_dtypes.bfloat16)),
             x=np.ascontiguousarray(x[b]),
             w01=w01, bias=bias, eyeb=eyeb, onesb=onesb, eyef=eyef, onef=onef)
        for b in range(B)
    ]


def kernel(adj, gcn_inputs, w_mean, b_mean, w1, b1, w2, b2):
    from concourse.bass_utils import run_bass_kernel_spmd

    nc = _get_nc()
    in_maps = _make_in_maps(adj, gcn_inputs, w_mean, b_mean, w1, b1, w2, b2)
    res = run_bass_kernel_spmd(nc, in_maps, core_ids=list(range(B)))
    mv = np.stack([res.results[b]["mv"] for b in range(B)])
    hid = np.stack([res.results[b]["hid"] for b in range(B)])
    out = np.stack([res.results[b]["out"] for b in range(B)])
    x = np.asarray(gcn_inputs, dtype=np.float32)
    return ((x, mv, hid, out), ())
